# revision 1
# baseline (speedup 1.0000x reference)
"""TAGConvNet (2x TAGConv K=3 + MLP) on 8 trn2 NeuronCores via Bass/Tile.

Strategy: node-partition across 8 cores (12544 padded rows each, 98 blocks of
128). Message passing per hop: dma_gather rows of the dis-scaled feature table
z = dis * x_k (replicated via AllGather), scatter-add via one-hot matmuls into
PSUM per 128-node block, then per-node scaling:
  x_{k+1} = dis * sum_{e: col=i} z[row_e],   z_{k+1} = dis^2 * sum(...).
Dense layer matmuls run in transposed orientation (features on partitions).
"""
import sys
from contextlib import ExitStack

import numpy as np

sys.path.insert(0, "/opt/trn_rl_repo")

import concourse.bass as bass  # noqa: E402
import concourse.tile as tile  # noqa: E402
from concourse import bacc, mybir  # noqa: E402
from concourse.bass_utils import run_bass_kernel_spmd  # noqa: E402

P = 8                 # cores
NBLK = 98             # 128-node blocks per core
NB = NBLK * 128       # 12544 padded nodes per core
NTOT = P * NB         # 100352
SEG = 25088           # int16-safe gather segment (NTOT / 4)
NSEGS = NTOT // SEG   # 4
GBLK = 4              # blocks per psum group (1 PSUM bank per block acc)
MAXL = 2048           # max idxs per dma_gather call
DT = mybir.dt

_cache = {}
SKIP_AG = False       # ablation: skip AllGathers
SKIP_GATHER = False   # ablation: skip dma_gather calls
SKIP_SCATTER = False  # ablation: skip onehot+matmul scatter
ITERS = 1             # repeat whole network in-program (for timing slope)


def _host_prep(edge_index, n_real):
    """Bucket edges by (core, target block, source segment) with cross-core
    common padded counts; returns per-core idx/colrel streams + call plan."""
    npc = n_real // P  # 12500 real nodes per core
    row, col = edge_index[0].astype(np.int64), edge_index[1].astype(np.int64)

    deg = np.bincount(col, minlength=n_real)
    dis = np.where(deg > 0, 1.0 / np.sqrt(np.maximum(deg, 1.0)), 0.0).astype(np.float32)

    def to_gid(i):
        return (i // npc) * NB + (i % npc)

    rg, cg = to_gid(row), to_gid(col)
    dis_g = np.zeros(NTOT, np.float32)
    dis_g[to_gid(np.arange(n_real))] = dis

    core = cg // NB
    loc = cg - core * NB
    blk = loc >> 7
    seg = rg // SEG

    cnt = np.zeros((P, NBLK, NSEGS), np.int64)
    np.add.at(cnt, (core, blk, seg), 1)
    pbs = (128 * np.ceil(cnt.max(axis=0) / 128.0)).astype(np.int64)  # [NBLK, NSEGS]

    # stream layout: for each group of GBLK blocks: for s: for b in group: pbs[b,s]
    off = np.zeros((NBLK, NSEGS), np.int64)
    pos = 0
    groups = [list(range(g, min(g + GBLK, NBLK))) for g in range(0, NBLK, GBLK)]
    calls = []  # (stream_off, L, seg, [(block, nchunks), ...])
    for blocks in groups:
        for s in range(NSEGS):
            cur = None
            for b in blocks:
                n = int(pbs[b, s])
                if n == 0:
                    continue
                off[b, s] = pos
                if cur is not None and cur[1] + n <= MAXL:
                    cur[1] += n
                    cur[3].append((b, n // 128))
                else:
                    if cur is not None:
                        calls.append(tuple(cur))
                    cur = [pos, n, s, [(b, n // 128)]]
                pos += n
            if cur is not None:
                calls.append(tuple(cur))
    epad = pos

    # per-core padded streams
    key = (core * NBLK + blk) * NSEGS + seg
    order = np.argsort(key, kind="stable")
    key_s = key[order]
    first = np.searchsorted(key_s, key_s)  # first pos of each key run
    rank = np.arange(len(key_s)) - first
    dst = off[blk[order], seg[order]] + rank  # position in padded stream

    gidx = np.zeros((P, epad), np.int16)
    colrel = np.full((P, epad), -1.0, np.float32)
    gidx[core[order], dst] = (rg[order] - seg[order] * SEG).astype(np.int16)
    colrel[core[order], dst] = (loc[order] - blk[order] * 128).astype(np.float32)

    # device layouts
    idx16 = np.tile(gidx.reshape(P, epad // 16, 16).transpose(0, 2, 1), (1, 8, 1)).copy()
    colrel128 = colrel.reshape(P, epad // 128, 128).transpose(0, 2, 1).copy()
    dis_blk = dis_g.reshape(P, NBLK, 128).transpose(0, 2, 1).copy()  # [P,128,NBLK]
    return dict(epad=epad, calls=calls, idx16=idx16, colrel=colrel128,
                dis=dis_blk, dis2=dis_blk * dis_blk, npc=npc)


def _build(prep, n_g, k_hops, n_m):
    """Trace + compile the SPMD program. Returns (nc, input tensor names)."""
    epad = prep["epad"]
    calls = prep["calls"]
    nm1 = k_hops + 1  # weight mats per TAG layer

    nc = bacc.Bacc("TRN2", target_bir_lowering=False, debug=False, num_devices=P)

    xT_d = nc.dram_tensor("xT", [8, NB], DT.float32, kind="ExternalInput")
    idx_d = nc.dram_tensor("idx", [128, epad // 16], DT.int16, kind="ExternalInput")
    colrel_d = nc.dram_tensor("colrel", [128, epad // 128], DT.float32, kind="ExternalInput")
    dis_d = nc.dram_tensor("dis", [128, NBLK], DT.float32, kind="ExternalInput")
    dis2_d = nc.dram_tensor("dis2", [128, NBLK], DT.float32, kind="ExternalInput")
    w0_d = nc.dram_tensor("w0", [8, 128], DT.float32, kind="ExternalInput")
    b0_d = nc.dram_tensor("b0", [128, 1], DT.float32, kind="ExternalInput")
    wtag_d = nc.dram_tensor("wtag", [n_g * nm1, 128, 128], DT.float32, kind="ExternalInput")
    btag_d = nc.dram_tensor("btag", [128, n_g], DT.float32, kind="ExternalInput")
    wmlp_d = nc.dram_tensor("wmlp", [n_m, 128, 128], DT.float32, kind="ExternalInput")
    bmlp_d = nc.dram_tensor("bmlp", [128, n_m], DT.float32, kind="ExternalInput")
    w1_d = nc.dram_tensor("w1", [128, 1], DT.float32, kind="ExternalInput")
    b1_d = nc.dram_tensor("b1", [1, 1], DT.float32, kind="ExternalInput")
    y_d = nc.dram_tensor("y", [1, NB], DT.float32, kind="ExternalOutput")

    zin = [nc.dram_tensor(f"zin{i}", [NB, 128], DT.float32) for i in range(2)]
    ztab = [nc.dram_tensor(f"ztab{i}", [NTOT, 128], DT.float32, addr_space="Shared")
            for i in range(2)]
    rg = [list(range(P))]

    groups = [list(range(g, min(g + GBLK, NBLK))) for g in range(0, NBLK, GBLK)]
    last_chunk = {}
    for (c_off, L, s, segs) in calls:
        jj = 0
        for (b, nch) in segs:
            for _ in range(nch):
                last_chunk[b] = c_off // 128 + jj
                jj += 1

    with tile.TileContext(nc) as tc:
        with ExitStack() as ctx:
            const = ctx.enter_context(tc.tile_pool(name="const", bufs=1))
            big = ctx.enter_context(tc.tile_pool(name="big", bufs=1))
            mpool = ctx.enter_context(tc.tile_pool(name="msg", bufs=3))
            wpool = ctx.enter_context(tc.tile_pool(name="work", bufs=3))
            opool = ctx.enter_context(tc.tile_pool(name="oh", bufs=4))
            xpool = ctx.enter_context(tc.tile_pool(name="xt", bufs=3))
            pacc = ctx.enter_context(tc.tile_pool(name="pacc", bufs=1, space="PSUM"))
            paux = ctx.enter_context(tc.tile_pool(name="paux", bufs=2, space="PSUM"))
            pden = ctx.enter_context(tc.tile_pool(name="pden", bufs=2, space="PSUM"))
            # PSUM budget (8 banks): 4x acc [128,128] (bank each, bufs=1),
            # aux [128,2,128] 1 bank x2 bufs, dense [128,512] 1 bank x2 bufs.
            # NOTE: matmul start=True zeroes a full 2KB bank, so accumulators
            # that live across segment passes must each own a bank.

            # constants
            iota = const.tile([128, 128], DT.float32)
            nc.gpsimd.iota(iota[:], pattern=[[1, 128]], base=0, channel_multiplier=0,
                           allow_small_or_imprecise_dtypes=True)
            ident = const.tile([128, 128], DT.float32)
            nc.gpsimd.memset(ident[:], 0.0)
            nc.gpsimd.affine_select(ident[:], ident[:], pattern=[[-1, 128]],
                                    compare_op=mybir.AluOpType.not_equal, fill=1.0,
                                    base=0, channel_multiplier=1)

            idx_sb = const.tile([128, epad // 16], DT.int16)
            nc.sync.dma_start(idx_sb[:], idx_d[:])
            colrel_sb = const.tile([128, epad // 128], DT.float32)
            nc.sync.dma_start(colrel_sb[:], colrel_d[:])
            dis_sb = const.tile([128, NBLK], DT.float32)
            nc.sync.dma_start(dis_sb[:], dis_d[:])
            dis2_sb = const.tile([128, NBLK], DT.float32)
            nc.sync.dma_start(dis2_sb[:], dis2_d[:])

            w0_sb = const.tile([8, 128], DT.float32)
            nc.sync.dma_start(w0_sb[:], w0_d[:])
            b0_sb = const.tile([128, 1], DT.float32)
            nc.sync.dma_start(b0_sb[:], b0_d[:])
            wtag_sb = []
            for i in range(n_g * nm1):
                t = const.tile([128, 128], DT.float32, tag=f"wtag{i}")
                nc.sync.dma_start(t[:], wtag_d[i])
                wtag_sb.append(t)
            btag_sb = const.tile([128, n_g], DT.float32)
            nc.sync.dma_start(btag_sb[:], btag_d[:])
            wmlp_sb = []
            for i in range(n_m):
                t = const.tile([128, 128], DT.float32, tag=f"wmlp{i}")
                nc.sync.dma_start(t[:], wmlp_d[i])
                wmlp_sb.append(t)
            bmlp_sb = const.tile([128, n_m], DT.float32)
            nc.sync.dma_start(bmlp_sb[:], bmlp_d[:])
            w1_sb = const.tile([128, 1], DT.float32)
            nc.sync.dma_start(w1_sb[:], w1_d[:])
            b1_sb = const.tile([1, 1], DT.float32)
            nc.sync.dma_start(b1_sb[:], b1_d[:])

            hT = big.tile([128, NB], DT.float32)    # h transposed [C, nodes]
            oT = big.tile([128, NB], DT.float32)    # out accumulator, same layout

            for _it in range(ITERS):
                # ---- lin0: hT = relu(W0^T xT + b0), batched 4 blocks ----
              for bb in range(0, NBLK, 4):
                  w = min(4, NBLK - bb) * 128
                  xt = xpool.tile([8, 512], DT.float32, tag="xt")
                  nc.sync.dma_start(xt[:, :w], xT_d[:, 128 * bb:128 * bb + w])
                  ph = pden.tile([128, 512], DT.float32, tag="ph")
                  nc.tensor.matmul(ph[:, :w], w0_sb[:], xt[:, :w])
                  nc.scalar.activation(hT[:, 128 * bb:128 * bb + w], ph[:, :w],
                                       mybir.ActivationFunctionType.Relu, bias=b0_sb[:])

              par = 0
              rel = mybir.ActivationFunctionType.Relu
              cpy = mybir.ActivationFunctionType.Copy

              for g in range(n_g):
                  # z0 = dis * h (row-major) -> zin[par]; out = W[g,0]^T h
                  for b in range(NBLK):
                      aux0 = paux.tile([128, 2, 128], DT.float32,
                                       name=f"aux0_{g}_{b}", tag="aux")
                      nc.tensor.transpose(aux0[:, 0, :], hT[:, 128 * b:128 * (b + 1)], ident[:])
                      zr = wpool.tile([128, 128], DT.float32, tag="zr")
                      nc.scalar.activation(zr[:], aux0[:, 0, :], cpy, scale=dis_sb[:, b:b + 1])
                      nc.sync.dma_start(zin[par][128 * b:128 * (b + 1), :], zr[:])
                  for bb in range(0, NBLK, 4):
                      w = min(4, NBLK - bb) * 128
                      po = pden.tile([128, 512], DT.float32, tag="ph")
                      nc.tensor.matmul(po[:, :w], wtag_sb[g * nm1][:],
                                       hT[:, 128 * bb:128 * bb + w])
                      nc.vector.tensor_copy(oT[:, 128 * bb:128 * bb + w], po[:, :w])
                  if not SKIP_AG:
                      nc.gpsimd.collective_compute(
                          "AllGather", mybir.AluOpType.bypass, replica_groups=rg,
                          ins=[zin[par][:]], outs=[ztab[par][:]])

                  for k in range(1, k_hops + 1):
                      nxt = par ^ 1
                      started = set()
                      for gi, blocks in enumerate(groups):
                          accs = {b: pacc.tile([128, 128], DT.float32,
                                               name=f"acc_{g}_{k}_{b}",
                                               tag=f"acc{b - blocks[0]}")
                                  for b in blocks}
                          for (c_off, L, s, segs) in calls:
                              if segs[0][0] not in accs:
                                  continue
                              msg = mpool.tile([128, MAXL // 128, 128], DT.float32, tag="msg")
                              if not SKIP_GATHER:
                                  nc.gpsimd.dma_gather(
                                      out_ap=msg[:, :L // 128, :],
                                      in_ap=ztab[par][s * SEG:(s + 1) * SEG, :],
                                      idxs_ap=idx_sb[:, c_off // 16:(c_off + L) // 16],
                                      num_idxs=L, num_idxs_reg=L, elem_size=128)
                              jj = 0
                              for (b, nch) in segs:
                                  last = last_chunk[b]
                                  for t in range(nch):
                                      j = c_off // 128 + jj
                                      first = b not in started
                                      if first:
                                          started.add(b)
                                      if SKIP_SCATTER:
                                          if first:
                                              nc.vector.memset(accs[b], 0.0)
                                      else:
                                          oh = opool.tile([128, 128], DT.float32, tag="oh")
                                          nc.vector.tensor_scalar(
                                              oh[:], iota[:], colrel_sb[:, j:j + 1], None,
                                              op0=mybir.AluOpType.is_equal)
                                          nc.tensor.matmul(accs[b], oh[:], msg[:, jj, :],
                                                           start=first,
                                                           stop=(j == last))
                                      jj += 1
                          # finalize blocks of this group
                          for b in blocks:
                              if b not in started:  # no edges at all: zeros
                                  nc.vector.memset(accs[b], 0.0)
                              if k < k_hops:
                                  zr = wpool.tile([128, 128], DT.float32, tag="zr")
                                  nc.scalar.activation(zr[:], accs[b], cpy,
                                                       scale=dis2_sb[:, b:b + 1])
                                  nc.sync.dma_start(zin[nxt][128 * b:128 * (b + 1), :], zr[:])
                              xk = wpool.tile([128, 128], DT.float32, tag="xk")
                              nc.vector.tensor_scalar(xk[:], accs[b], dis_sb[:, b:b + 1],
                                                      None, op0=mybir.AluOpType.mult)
                              aux = paux.tile([128, 2, 128], DT.float32,
                                              name=f"aux_{g}_{k}_{b}", tag="aux")
                              nc.tensor.transpose(aux[:, 0, :], xk[:], ident[:])
                              xkT = wpool.tile([128, 128], DT.float32, tag="xkT")
                              nc.scalar.activation(xkT[:], aux[:, 0, :], cpy)
                              nc.tensor.matmul(aux[:, 1, :], wtag_sb[g * nm1 + k][:], xkT[:])
                              nc.vector.tensor_add(oT[:, 128 * b:128 * (b + 1)],
                                                   oT[:, 128 * b:128 * (b + 1)], aux[:, 1, :])
                      if k < k_hops:
                          if not SKIP_AG:
                              nc.gpsimd.collective_compute(
                                  "AllGather", mybir.AluOpType.bypass, replica_groups=rg,
                                  ins=[zin[nxt][:]], outs=[ztab[nxt][:]])
                          par = nxt

                  # layer end: h = relu(out + b_tag[g]) in place, then swap buffers
                  nc.scalar.activation(oT[:], oT[:], rel, bias=btag_sb[:, g:g + 1])
                  hT, oT = oT, hT

              # ---- MLP ----
              for m in range(n_m):
                  for bb in range(0, NBLK, 4):
                      w = min(4, NBLK - bb) * 128
                      po = pden.tile([128, 512], DT.float32, tag="ph")
                      nc.tensor.matmul(po[:, :w], wmlp_sb[m][:],
                                       hT[:, 128 * bb:128 * bb + w])
                      nc.scalar.activation(oT[:, 128 * bb:128 * bb + w], po[:, :w],
                                           rel, bias=bmlp_sb[:, m:m + 1])
                  hT, oT = oT, hT

              # ---- head: y = relu(W1^T h + b1) ----
              ysb = big.tile([1, NB], DT.float32)
              for bb in range(0, NBLK, 4):
                  w = min(4, NBLK - bb) * 128
                  py = pden.tile([1, 512], DT.float32, tag="ph")
                  nc.tensor.matmul(py[:, :w], w1_sb[:], hT[:, 128 * bb:128 * bb + w])
                  nc.scalar.activation(ysb[:, 128 * bb:128 * bb + w], py[:, :w],
                                       rel, bias=b1_sb[:])
              nc.sync.dma_start(y_d[:], ysb[:])

    nc.compile()
    return nc


def _setup(x, edge_index, W0, b0, W_tag, b_tag, W_mlp, b_mlp, W1, b1):
    x = np.asarray(x, np.float32)
    edge_index = np.asarray(edge_index)
    n_real = x.shape[0]
    n_g, nm1 = W_tag.shape[0], W_tag.shape[1]
    n_m = W_mlp.shape[0]

    ck = (n_real, edge_index.shape[1], int(edge_index[0, ::997].astype(np.int64).sum()),
          int(edge_index[1, ::997].astype(np.int64).sum()))
    if ck not in _cache:
        prep = _host_prep(edge_index, n_real)
        nc = _build(prep, n_g, nm1 - 1, n_m)
        _cache[ck] = (prep, nc)
    prep, nc = _cache[ck]

    npc = prep["npc"]
    # per-core transposed x, padded [8, NB]
    xT = np.zeros((P, 8, NB), np.float32)
    xs = x.reshape(P, npc, -1)
    for c in range(P):
        xT[c, :xs.shape[2], :npc] = xs[c].T

    wtag = np.ascontiguousarray(W_tag.reshape(n_g * nm1, 128, 128), dtype=np.float32)
    in_maps = []
    for c in range(P):
        in_maps.append({
            "xT": xT[c], "idx": prep["idx16"][c], "colrel": prep["colrel"][c],
            "dis": prep["dis"][c], "dis2": prep["dis2"][c],
            "w0": np.vstack([np.asarray(W0, np.float32),
                             np.zeros((8 - W0.shape[0], 128), np.float32)]),
            "b0": np.asarray(b0, np.float32).reshape(128, 1),
            "wtag": wtag,
            "btag": np.ascontiguousarray(np.asarray(b_tag, np.float32).T),
            "wmlp": np.asarray(W_mlp, np.float32),
            "bmlp": np.ascontiguousarray(np.asarray(b_mlp, np.float32).T),
            "w1": np.asarray(W1, np.float32),
            "b1": np.asarray(b1, np.float32).reshape(1, 1),
        })
    return nc, in_maps, npc, n_real


def kernel(**inputs):
    nc, in_maps, npc, n_real = _setup(**inputs)
    res = run_bass_kernel_spmd(nc, in_maps, list(range(P)))
    out = np.concatenate([res.results[c]["y"][0, :npc] for c in range(P)])
    return out.reshape(n_real, 1).astype(np.float32)


def run_traced(inputs):
    nc, in_maps, npc, n_real = _setup(**inputs)
    return run_bass_kernel_spmd(nc, in_maps, list(range(P)), trace=True)



# revision 3
# speedup vs baseline: 1.0048x; 1.0048x over previous
"""TAGConvNet (2x TAGConv K=3 + MLP) on 8 trn2 NeuronCores via Bass/Tile. v2.

Strategy (node-partition, 12800 padded rows/core = 100 blocks of 128):
- Message table z (= dis*x_k) replicated via per-quarter AllGathers in bf16.
  Quarter q = rows [3200q, 3200q+3200) of every core; quarter table is
  [8*3200=25600, 128] bf16 (int16-indexable).
- Per hop: for each group of 4 target blocks, gather padded per-(block,
  quarter) edge-source rows (bf16, 256B rows), build 4-wide one-hot tiles
  with a single broadcast tensor_tensor per quad, matmul-accumulate into a
  packed PSUM bank [128, 4, 128] (memzero + start=False).
- Finalize per block: zin(bf16) = dis2*acc -> DMA -> quarter AllGather
  (pipelined, fires when its 25 blocks are done); xk = dis*acc (bf16),
  transpose via PE, out^T += W_k^T xk^T.
- Dense layers (lin0 / W_g0 / MLP / head) run feature-major in fp32.
"""
import sys
from contextlib import ExitStack

import numpy as np

sys.path.insert(0, "/opt/trn_rl_repo")

import ml_dtypes  # noqa: E402
import concourse.bass as bass  # noqa: E402
import concourse.tile as tile  # noqa: E402
from concourse import bacc, mybir  # noqa: E402
from concourse.bass_utils import run_bass_kernel_spmd  # noqa: E402

P = 8                  # cores
NBLK = 100             # 128-node blocks per core
NB = NBLK * 128        # 12800 padded nodes per core
NTOT = P * NB          # 102400
NQ = 4                 # source quarters
QN = NB // NQ          # 3200 nodes per quarter per core
QSEG = P * QN          # 25600 rows per quarter gather table
GBLK = 4               # blocks per group (packed psum bank)
MAXL = 1024            # max idxs per dma_gather call (2048 wedges the runtime)
DT = mybir.dt
BF16 = DT.bfloat16
NQUEUES = 1            # SWDGE queues to round-robin gathers over

_cache = {}


def _host_prep(edge_index, n_real):
    npc = n_real // P  # 12500 real nodes per core
    row, col = edge_index[0].astype(np.int64), edge_index[1].astype(np.int64)

    deg = np.bincount(col, minlength=n_real)
    dis = np.where(deg > 0, 1.0 / np.sqrt(np.maximum(deg, 1.0)), 0.0).astype(np.float32)

    core_s, loc_s = row // npc, row % npc
    core_t, loc_t = col // npc, col % npc
    blk = loc_t >> 7                      # target block 0..97
    q = loc_s // QN                       # source quarter 0..3
    srow = core_s * QN + (loc_s - q * QN)  # row in quarter table

    cnt = np.zeros((P, NBLK, NQ), np.int64)
    np.add.at(cnt, (core_t, blk, q), 1)
    pbs = (128 * np.ceil(cnt.max(axis=0) / 128.0)).astype(np.int64)  # [NBLK, NQ]

    groups = [list(range(g, min(g + GBLK, NBLK))) for g in range(0, NBLK, GBLK)]
    off = np.zeros((NBLK, NQ), np.int64)
    pos = 0
    calls = []  # (group_idx, q, stream_off, L, [(block, nchunks), ...])
    for gi, blocks in enumerate(groups):
        for qq in range(NQ):
            cur = None
            for b in blocks:
                n = int(pbs[b, qq])
                if n == 0:
                    continue
                off[b, qq] = pos
                if cur is not None and cur[3] + n <= MAXL:
                    cur[3] += n
                    cur[4].append((b, n // 128))
                else:
                    if cur is not None:
                        calls.append(tuple(cur))
                    cur = [gi, qq, pos, n, [(b, n // 128)]]
                pos += n
            if cur is not None:
                calls.append(tuple(cur))
    epad = pos

    # slot assignment
    key = (core_t * NBLK + blk) * NQ + q
    order = np.argsort(key, kind="stable")
    key_s = key[order]
    first = np.searchsorted(key_s, key_s)
    rank = np.arange(len(key_s)) - first
    dst = off[blk[order], q[order]] + rank

    gidx = np.zeros((P, epad), np.int16)
    colrel = np.full((P, epad), -1.0, np.float32)
    gidx[core_t[order], dst] = srow[order].astype(np.int16)
    colrel[core_t[order], dst] = (loc_t[order] - blk[order] * 128).astype(np.float32)

    dis_g = np.zeros((P, NB), np.float32)
    dis_g[:, :npc] = dis.reshape(P, npc)

    idx16 = np.tile(gidx.reshape(P, epad // 16, 16).transpose(0, 2, 1), (1, 8, 1)).copy()
    colrel128 = np.ascontiguousarray(
        colrel.reshape(P, epad // 128, 128).transpose(0, 2, 1)
    ).astype(ml_dtypes.bfloat16)  # [P, 128, NCHUNK]
    dis_blk = np.ascontiguousarray(
        dis_g.reshape(P, NBLK, 128).transpose(0, 2, 1))  # [P, 128, NBLK]
    return dict(epad=epad, calls=calls, idx16=idx16, colrel=colrel128,
                dis=dis_blk, dis2=dis_blk * dis_blk, npc=npc)


def _build(prep, n_g, k_hops, n_m):
    epad = prep["epad"]
    calls = prep["calls"]
    nm1 = k_hops + 1
    nchunk = epad // 128

    # last chunk (global chunk id) per block, for matmul stop flags
    last_chunk = {}
    chunks_of_block = {}
    for (gi, qq, c_off, L, segs) in calls:
        j = c_off // 128
        for (b, nch) in segs:
            for t in range(nch):
                last_chunk[b] = j
                chunks_of_block.setdefault(b, []).append(j)
                j += 1

    nc = bacc.Bacc("TRN2", target_bir_lowering=False, debug=False,
                   num_devices=P, num_swdge_queues=4)

    xT_d = nc.dram_tensor("xT", [8, NB], DT.float32, kind="ExternalInput")
    idx_d = nc.dram_tensor("idx", [128, epad // 16], DT.int16, kind="ExternalInput")
    colrel_d = nc.dram_tensor("colrel", [128, nchunk], BF16, kind="ExternalInput")
    dis_d = nc.dram_tensor("dis", [128, NBLK], DT.float32, kind="ExternalInput")
    dis2_d = nc.dram_tensor("dis2", [128, NBLK], DT.float32, kind="ExternalInput")
    w0_d = nc.dram_tensor("w0", [8, 128], DT.float32, kind="ExternalInput")
    b0_d = nc.dram_tensor("b0", [128, 1], DT.float32, kind="ExternalInput")
    wtag_d = nc.dram_tensor("wtag", [n_g * nm1, 128, 128], DT.float32, kind="ExternalInput")
    wtagh_d = nc.dram_tensor("wtagh", [n_g * nm1, 128, 128], BF16, kind="ExternalInput")
    btag_d = nc.dram_tensor("btag", [128, n_g], DT.float32, kind="ExternalInput")
    wmlp_d = nc.dram_tensor("wmlp", [n_m, 128, 128], DT.float32, kind="ExternalInput")
    bmlp_d = nc.dram_tensor("bmlp", [128, n_m], DT.float32, kind="ExternalInput")
    w1_d = nc.dram_tensor("w1", [128, 1], DT.float32, kind="ExternalInput")
    b1_d = nc.dram_tensor("b1", [1, 1], DT.float32, kind="ExternalInput")
    y_d = nc.dram_tensor("y", [1, NB], DT.float32, kind="ExternalOutput")

    zin = [[nc.dram_tensor(f"zin{par}_{qq}", [QN, 128], BF16)
            for qq in range(NQ)] for par in range(2)]
    ztab = [[nc.dram_tensor(f"ztab{par}_{qq}", [QSEG, 128], BF16, addr_space="Shared")
             for qq in range(NQ)] for par in range(2)]
    rg = [list(range(P))]

    groups = [list(range(g, min(g + GBLK, NBLK))) for g in range(0, NBLK, GBLK)]
    calls_of_group = {}
    for c in calls:
        calls_of_group.setdefault((c[0], c[1]), []).append(c)

    rel = mybir.ActivationFunctionType.Relu
    cpy = mybir.ActivationFunctionType.Copy

    with tile.TileContext(nc) as tc:
        with ExitStack() as ctx:
            const = ctx.enter_context(tc.tile_pool(name="const", bufs=1))
            big = ctx.enter_context(tc.tile_pool(name="big", bufs=1))
            mpool = ctx.enter_context(tc.tile_pool(name="msg", bufs=6))
            opool = ctx.enter_context(tc.tile_pool(name="oh", bufs=8))
            wpool = ctx.enter_context(tc.tile_pool(name="work", bufs=6))
            xpool = ctx.enter_context(tc.tile_pool(name="xt", bufs=3))
            pacc = ctx.enter_context(tc.tile_pool(name="pacc", bufs=3, space="PSUM"))
            paux = ctx.enter_context(tc.tile_pool(name="paux", bufs=2, space="PSUM"))
            pden = ctx.enter_context(tc.tile_pool(name="pden", bufs=2, space="PSUM"))

            # ---- constants ----
            iota = const.tile([128, 128], DT.float32)
            nc.gpsimd.iota(iota[:], pattern=[[1, 128]], base=0, channel_multiplier=0,
                           allow_small_or_imprecise_dtypes=True)
            iota_h = const.tile([128, 128], BF16)
            nc.vector.tensor_copy(iota_h[:], iota[:])
            ident = const.tile([128, 128], DT.float32)
            nc.gpsimd.memset(ident[:], 0.0)
            nc.gpsimd.affine_select(ident[:], ident[:], pattern=[[-1, 128]],
                                    compare_op=mybir.AluOpType.not_equal, fill=1.0,
                                    base=0, channel_multiplier=1)
            ident_h = const.tile([128, 128], BF16)
            nc.vector.tensor_copy(ident_h[:], ident[:])

            idx_sb = const.tile([128, epad // 16], DT.int16)
            nc.sync.dma_start(idx_sb[:], idx_d[:])
            colrel_sb = const.tile([128, nchunk], BF16)
            nc.sync.dma_start(colrel_sb[:], colrel_d[:])
            dis_sb = const.tile([128, NBLK], DT.float32)
            nc.sync.dma_start(dis_sb[:], dis_d[:])
            dis2_sb = const.tile([128, NBLK], DT.float32)
            nc.sync.dma_start(dis2_sb[:], dis2_d[:])

            w0_sb = const.tile([8, 128], DT.float32)
            nc.sync.dma_start(w0_sb[:], w0_d[:])
            b0_sb = const.tile([128, 1], DT.float32)
            nc.sync.dma_start(b0_sb[:], b0_d[:])
            wtag_sb = []   # fp32, used for k=0 dense matmul
            wtagh_sb = []  # bf16, used for k>=1
            for i in range(n_g * nm1):
                if i % nm1 == 0:
                    t = const.tile([128, 128], DT.float32, tag=f"wtag{i}")
                    nc.sync.dma_start(t[:], wtag_d[i])
                    wtag_sb.append(t)
                    wtagh_sb.append(None)
                else:
                    t = const.tile([128, 128], BF16, tag=f"wtagh{i}")
                    nc.sync.dma_start(t[:], wtagh_d[i])
                    wtag_sb.append(None)
                    wtagh_sb.append(t)
            btag_sb = const.tile([128, n_g], DT.float32)
            nc.sync.dma_start(btag_sb[:], btag_d[:])
            wmlp_sb = []
            for i in range(n_m):
                t = const.tile([128, 128], DT.float32, tag=f"wmlp{i}")
                nc.sync.dma_start(t[:], wmlp_d[i])
                wmlp_sb.append(t)
            bmlp_sb = const.tile([128, n_m], DT.float32)
            nc.sync.dma_start(bmlp_sb[:], bmlp_d[:])
            w1_sb = const.tile([128, 1], DT.float32)
            nc.sync.dma_start(w1_sb[:], w1_d[:])
            b1_sb = const.tile([1, 1], DT.float32)
            nc.sync.dma_start(b1_sb[:], b1_d[:])

            hT = big.tile([128, NB], DT.float32)
            oT = big.tile([128, NB], DT.float32)

            # ---- lin0: hT = relu(W0^T xT + b0), emitted per slab inside the
            # layer-0 z0 walk so quarter-0's AllGather fires early ----
            lin0_done = set()

            def emit_lin0_slab(s):
                bb = 4 * s
                xt = xpool.tile([8, 512], DT.float32, tag="xt")
                nc.sync.dma_start(xt[:], xT_d[:, 128 * bb:128 * bb + 512])
                ph = pden.tile([128, 512], DT.float32, tag="ph")
                nc.tensor.matmul(ph[:], w0_sb[:], xt[:])
                nc.scalar.activation(hT[:, 128 * bb:128 * bb + 512], ph[:],
                                     rel, bias=b0_sb[:])

            par = 0
            qrr = [0]  # gather queue round-robin counter

            def quarter_of_block(b):
                return b // (NBLK // NQ)

            def emit_z0_block(b, zpar):
                # z0 = dis*h for block b -> zin[zpar]
                aux0 = paux.tile([128, 2, 128], DT.float32, tag="aux")
                nc.tensor.transpose(aux0[:, 0, :], hT[:, 128 * b:128 * (b + 1)], ident[:])
                zr = wpool.tile([128, 128], BF16, tag="zr")
                nc.scalar.activation(zr[:], aux0[:, 0, :], cpy, scale=dis_sb[:, b:b + 1])
                qq = quarter_of_block(b)
                lb = b - qq * (NBLK // NQ)
                nc.sync.dma_start(zin[zpar][qq][128 * lb:128 * (lb + 1), :], zr[:])

            def emit_ag(qq, zpar):
                nc.gpsimd.collective_compute(
                    "AllGather", mybir.AluOpType.bypass, replica_groups=rg,
                    ins=[zin[zpar][qq][:]], outs=[ztab[zpar][qq][:]])

            for g in range(n_g):
                # z0 blocks + quarter AGs (only blocks 0..97 hold real nodes)
                for qq in range(NQ):
                    for b in range(qq * 25, min(qq * 25 + 25, 98)):
                        s = b // 4
                        if g == 0 and s not in lin0_done:
                            lin0_done.add(s)
                            emit_lin0_slab(s)
                        emit_z0_block(b, par)
                    emit_ag(qq, par)
                if g == 0:
                    for s in range(NBLK // 4):
                        if s not in lin0_done:
                            lin0_done.add(s)
                            emit_lin0_slab(s)

                # out^T init: W[g,0]^T hT
                for bb in range(0, NBLK, 4):
                    po = pden.tile([128, 512], DT.float32, tag="ph")
                    nc.tensor.matmul(po[:], wtag_sb[g * nm1][:],
                                     hT[:, 128 * bb:128 * bb + 512])
                    nc.vector.tensor_copy(oT[:, 128 * bb:128 * bb + 512], po[:])

                for k in range(1, k_hops + 1):
                    nxt = par ^ 1
                    ag_fired = set()
                    for gi, blocks in enumerate(groups):
                        gcalls = [c for qq in range(NQ)
                                  for c in calls_of_group.get((gi, qq), [])]
                        if not gcalls:
                            continue  # pure-pad block group
                        acc = pacc.tile([128, GBLK, 128], DT.float32,
                                        name=f"acc_{g}_{k}_{gi}", tag="acc")
                        nc.scalar.memzero(acc[:])
                        for (gi_, qq, c_off, L, segs) in gcalls:
                            msg = mpool.tile([128, MAXL // 128, 128], BF16, tag="msg")
                            nc.gpsimd.dma_gather(
                                out_ap=msg[:, :L // 128, :],
                                in_ap=ztab[par][qq][:, :],
                                idxs_ap=idx_sb[:, c_off // 16:(c_off + L) // 16],
                                num_idxs=L, num_idxs_reg=L, elem_size=128,
                                queue_num=qrr[0] % NQUEUES)
                            qrr[0] += 1
                            # chunk -> block map for this call
                            cblocks = [b for (b, nch) in segs for _ in range(nch)]
                            nch = L // 128
                            c0 = c_off // 128
                            for q0 in range(0, nch, 4):
                                w = min(4, nch - q0)
                                oh = opool.tile([128, 4, 128], BF16, tag="oh")
                                in0 = iota_h[:].unsqueeze(1).broadcast_to([128, w, 128])
                                in1 = colrel_sb[:, c0 + q0:c0 + q0 + w].unsqueeze(2) \
                                    .broadcast_to([128, w, 128])
                                nc.vector.tensor_tensor(oh[:, :w, :], in0, in1,
                                                        op=mybir.AluOpType.is_equal)
                                for j in range(w):
                                    cj = q0 + j
                                    b = cblocks[cj]
                                    nc.tensor.matmul(
                                        acc[:, b - blocks[0], :], oh[:, j, :],
                                        msg[:, cj, :], start=False,
                                        stop=(c0 + cj == last_chunk[b]))
                        # finalize the group's blocks
                        for b in blocks:
                            if b not in chunks_of_block:
                                continue
                            jj = b - blocks[0]
                            if k < k_hops:
                                zr = wpool.tile([128, 128], BF16, tag="zr")
                                nc.scalar.activation(zr[:], acc[:, jj, :], cpy,
                                                     scale=dis2_sb[:, b:b + 1])
                                qq2 = quarter_of_block(b)
                                lb = b - qq2 * 25
                                nc.sync.dma_start(
                                    zin[nxt][qq2][128 * lb:128 * (lb + 1), :], zr[:])
                            xk = wpool.tile([128, 128], DT.float32, tag="xk")
                            nc.vector.tensor_scalar(xk[:], acc[:, jj, :],
                                                    dis_sb[:, b:b + 1], None,
                                                    op0=mybir.AluOpType.mult)
                            aux = paux.tile([128, 2, 128], DT.float32,
                                            name=f"aux_{g}_{k}_{b}", tag="aux")
                            nc.tensor.transpose(aux[:, 0, :], xk[:], ident[:])
                            xkT = wpool.tile([128, 128], BF16, tag="xkT")
                            nc.scalar.activation(xkT[:], aux[:, 0, :], cpy)
                            nc.tensor.matmul(aux[:, 1, :], wtagh_sb[g * nm1 + k][:],
                                             xkT[:])
                            nc.vector.tensor_add(oT[:, 128 * b:128 * (b + 1)],
                                                 oT[:, 128 * b:128 * (b + 1)],
                                                 aux[:, 1, :])
                        # fire AG for every quarter fully finalized by now
                        if k < k_hops:
                            for qq2 in range(NQ):
                                if qq2 not in ag_fired and blocks[-1] >= qq2 * 25 + 24:
                                    ag_fired.add(qq2)
                                    emit_ag(qq2, nxt)
                    if k < k_hops:
                        assert ag_fired == set(range(NQ))
                        par = nxt

                # layer end: h = relu(out + b_tag[g]), per 4-block slab so each
                # slab unblocks as soon as its hop-3 finalize lands
                for bb in range(0, NBLK, 4):
                    nc.scalar.activation(hT[:, 128 * bb:128 * bb + 512],
                                         oT[:, 128 * bb:128 * bb + 512],
                                         rel, bias=btag_sb[:, g:g + 1])

            # ---- MLP ----
            for m in range(n_m):
                src_t, dst_t = (hT, oT) if m % 2 == 0 else (oT, hT)
                for bb in range(0, NBLK, 4):
                    po = pden.tile([128, 512], DT.float32, tag="ph")
                    nc.tensor.matmul(po[:], wmlp_sb[m][:],
                                     src_t[:, 128 * bb:128 * bb + 512])
                    nc.scalar.activation(dst_t[:, 128 * bb:128 * bb + 512], po[:],
                                         rel, bias=bmlp_sb[:, m:m + 1])
            hT = oT if n_m % 2 == 1 else hT

            # ---- head ----
            ysb = big.tile([1, NB], DT.float32)
            for bb in range(0, NBLK, 4):
                py = pden.tile([1, 512], DT.float32, tag="ph")
                nc.tensor.matmul(py[:], w1_sb[:], hT[:, 128 * bb:128 * bb + 512])
                nc.scalar.activation(ysb[:, 128 * bb:128 * bb + 512], py[:],
                                     rel, bias=b1_sb[:])
                nc.sync.dma_start(y_d[:, 128 * bb:128 * bb + 512],
                                  ysb[:, 128 * bb:128 * bb + 512])

    nc.compile()
    return nc


def _setup(x, edge_index, W0, b0, W_tag, b_tag, W_mlp, b_mlp, W1, b1):
    x = np.asarray(x, np.float32)
    edge_index = np.asarray(edge_index)
    n_real = x.shape[0]
    n_g, nm1 = W_tag.shape[0], W_tag.shape[1]
    n_m = W_mlp.shape[0]

    ck = (n_real, edge_index.shape[1], int(edge_index[0, ::997].astype(np.int64).sum()),
          int(edge_index[1, ::997].astype(np.int64).sum()))
    if ck not in _cache:
        prep = _host_prep(edge_index, n_real)
        nc = _build(prep, n_g, nm1 - 1, n_m)
        _cache[ck] = (prep, nc)
    prep, nc = _cache[ck]

    npc = prep["npc"]
    xT = np.zeros((P, 8, NB), np.float32)
    xs = x.reshape(P, npc, -1)
    for c in range(P):
        xT[c, :xs.shape[2], :npc] = xs[c].T

    wtag = np.ascontiguousarray(W_tag.reshape(n_g * nm1, 128, 128), dtype=np.float32)
    in_maps = []
    for c in range(P):
        in_maps.append({
            "xT": xT[c], "idx": prep["idx16"][c], "colrel": prep["colrel"][c],
            "dis": prep["dis"][c], "dis2": prep["dis2"][c],
            "w0": np.vstack([np.asarray(W0, np.float32),
                             np.zeros((8 - W0.shape[0], 128), np.float32)]),
            "b0": np.asarray(b0, np.float32).reshape(128, 1),
            "wtag": wtag,
            "wtagh": wtag.astype(ml_dtypes.bfloat16),
            "btag": np.ascontiguousarray(np.asarray(b_tag, np.float32).T),
            "wmlp": np.asarray(W_mlp, np.float32),
            "bmlp": np.ascontiguousarray(np.asarray(b_mlp, np.float32).T),
            "w1": np.asarray(W1, np.float32),
            "b1": np.asarray(b1, np.float32).reshape(1, 1),
        })
    return nc, in_maps, npc, n_real


def kernel(**inputs):
    nc, in_maps, npc, n_real = _setup(**inputs)
    res = run_bass_kernel_spmd(nc, in_maps, list(range(P)))
    out = np.concatenate([res.results[c]["y"][0, :npc] for c in range(P)])
    return out.reshape(n_real, 1).astype(np.float32)


def run_traced(inputs):
    nc, in_maps, npc, n_real = _setup(**inputs)
    return run_bass_kernel_spmd(nc, in_maps, list(range(P)), trace=True)


# revision 6
# speedup vs baseline: 1.0059x; 1.0011x over previous
"""TAGConvNet (2x TAGConv K=3 + MLP) on 8 trn2 NeuronCores via Bass/Tile. v2.

Strategy (node-partition, 12800 padded rows/core = 100 blocks of 128):
- Message table z (= dis*x_k) replicated via per-quarter AllGathers in bf16.
  Quarter q = rows [3200q, 3200q+3200) of every core; quarter table is
  [8*3200=25600, 128] bf16 (int16-indexable).
- Per hop: for each group of 4 target blocks, gather padded per-(block,
  quarter) edge-source rows (bf16, 256B rows), build 4-wide one-hot tiles
  with a single broadcast tensor_tensor per quad, matmul-accumulate into a
  packed PSUM bank [128, 4, 128] (memzero + start=False).
- Finalize per block: zin(bf16) = dis2*acc -> DMA -> quarter AllGather
  (pipelined, fires when its 25 blocks are done); xk = dis*acc (bf16),
  transpose via PE, out^T += W_k^T xk^T.
- Dense layers (lin0 / W_g0 / MLP / head) run feature-major in fp32.
"""
import sys
from contextlib import ExitStack

import numpy as np

sys.path.insert(0, "/opt/trn_rl_repo")

import ml_dtypes  # noqa: E402
import concourse.bass as bass  # noqa: E402
import concourse.tile as tile  # noqa: E402
from concourse import bacc, mybir  # noqa: E402
from concourse.bass_utils import run_bass_kernel_spmd  # noqa: E402

P = 8                  # cores
NBLK = 100             # 128-node blocks per core
NB = NBLK * 128        # 12800 padded nodes per core
NTOT = P * NB          # 102400
NQ = 4                 # source quarters
QN = NB // NQ          # 3200 nodes per quarter per core
QSEG = P * QN          # 25600 rows per quarter gather table
GBLK = 4               # blocks per group (packed psum bank)
MAXL = 1024            # max idxs per dma_gather call (2048 wedges the runtime)
DT = mybir.dt
BF16 = DT.bfloat16
NQUEUES = 1            # SWDGE queues to round-robin gathers over

_cache = {}


def _host_prep(edge_index, n_real):
    npc = n_real // P  # 12500 real nodes per core
    row, col = edge_index[0].astype(np.int64), edge_index[1].astype(np.int64)

    deg = np.bincount(col, minlength=n_real)
    dis = np.where(deg > 0, 1.0 / np.sqrt(np.maximum(deg, 1.0)), 0.0).astype(np.float32)

    core_s, loc_s = row // npc, row % npc
    core_t, loc_t = col // npc, col % npc
    blk = loc_t >> 7                      # target block 0..97
    q = loc_s // QN                       # source quarter 0..3
    srow = core_s * QN + (loc_s - q * QN)  # row in quarter table

    cnt = np.zeros((P, NBLK, NQ), np.int64)
    np.add.at(cnt, (core_t, blk, q), 1)
    pbs = (128 * np.ceil(cnt.max(axis=0) / 128.0)).astype(np.int64)  # [NBLK, NQ]

    groups = [list(range(g, min(g + GBLK, NBLK))) for g in range(0, NBLK, GBLK)]
    off = np.zeros((NBLK, NQ), np.int64)
    pos = 0
    calls = []  # (group_idx, q, stream_off, L, [(block, nchunks), ...])
    for gi, blocks in enumerate(groups):
        for qq in range(NQ):
            cur = None
            for b in blocks:
                n = int(pbs[b, qq])
                if n == 0:
                    continue
                off[b, qq] = pos
                if cur is not None and cur[3] + n <= MAXL:
                    cur[3] += n
                    cur[4].append((b, n // 128))
                else:
                    if cur is not None:
                        calls.append(tuple(cur))
                    cur = [gi, qq, pos, n, [(b, n // 128)]]
                pos += n
            if cur is not None:
                calls.append(tuple(cur))
    epad = pos

    # slot assignment
    key = (core_t * NBLK + blk) * NQ + q
    order = np.argsort(key, kind="stable")
    key_s = key[order]
    first = np.searchsorted(key_s, key_s)
    rank = np.arange(len(key_s)) - first
    dst = off[blk[order], q[order]] + rank

    gidx = np.zeros((P, epad), np.int16)
    colrel = np.full((P, epad), -1.0, np.float32)
    gidx[core_t[order], dst] = srow[order].astype(np.int16)
    colrel[core_t[order], dst] = (loc_t[order] - blk[order] * 128).astype(np.float32)

    dis_g = np.zeros((P, NB), np.float32)
    dis_g[:, :npc] = dis.reshape(P, npc)

    idx16 = np.tile(gidx.reshape(P, epad // 16, 16).transpose(0, 2, 1), (1, 8, 1)).copy()
    colrel128 = np.ascontiguousarray(
        colrel.reshape(P, epad // 128, 128).transpose(0, 2, 1)
    ).astype(ml_dtypes.bfloat16)  # [P, 128, NCHUNK]
    dis_blk = np.ascontiguousarray(
        dis_g.reshape(P, NBLK, 128).transpose(0, 2, 1))  # [P, 128, NBLK]
    return dict(epad=epad, calls=calls, idx16=idx16, colrel=colrel128,
                dis=dis_blk, dis2=dis_blk * dis_blk, npc=npc)


def _build(prep, n_g, k_hops, n_m):
    epad = prep["epad"]
    calls = prep["calls"]
    nm1 = k_hops + 1
    nchunk = epad // 128

    # last chunk (global chunk id) per block, for matmul stop flags
    last_chunk = {}
    chunks_of_block = {}
    for (gi, qq, c_off, L, segs) in calls:
        j = c_off // 128
        for (b, nch) in segs:
            for t in range(nch):
                last_chunk[b] = j
                chunks_of_block.setdefault(b, []).append(j)
                j += 1

    nc = bacc.Bacc("TRN2", target_bir_lowering=False, debug=False,
                   num_devices=P, num_swdge_queues=4)

    xT_d = nc.dram_tensor("xT", [8, NB], DT.float32, kind="ExternalInput")
    idx_d = nc.dram_tensor("idx", [128, epad // 16], DT.int16, kind="ExternalInput")
    colrel_d = nc.dram_tensor("colrel", [128, nchunk], BF16, kind="ExternalInput")
    dis_d = nc.dram_tensor("dis", [128, NBLK], DT.float32, kind="ExternalInput")
    dis2_d = nc.dram_tensor("dis2", [128, NBLK], DT.float32, kind="ExternalInput")
    w0_d = nc.dram_tensor("w0", [8, 128], DT.float32, kind="ExternalInput")
    b0_d = nc.dram_tensor("b0", [128, 1], DT.float32, kind="ExternalInput")
    wtag_d = nc.dram_tensor("wtag", [n_g * nm1, 128, 128], DT.float32, kind="ExternalInput")
    wtagh_d = nc.dram_tensor("wtagh", [n_g * nm1, 128, 128], BF16, kind="ExternalInput")
    btag_d = nc.dram_tensor("btag", [128, n_g], DT.float32, kind="ExternalInput")
    wmlp_d = nc.dram_tensor("wmlp", [n_m, 128, 128], DT.float32, kind="ExternalInput")
    bmlp_d = nc.dram_tensor("bmlp", [128, n_m], DT.float32, kind="ExternalInput")
    w1_d = nc.dram_tensor("w1", [128, 1], DT.float32, kind="ExternalInput")
    b1_d = nc.dram_tensor("b1", [1, 1], DT.float32, kind="ExternalInput")
    y_d = nc.dram_tensor("y", [1, NB], DT.float32, kind="ExternalOutput")

    zin = [[nc.dram_tensor(f"zin{par}_{qq}", [QN, 128], BF16)
            for qq in range(NQ)] for par in range(2)]
    ztab = [[nc.dram_tensor(f"ztab{par}_{qq}", [QSEG, 128], BF16, addr_space="Shared")
             for qq in range(NQ)] for par in range(2)]
    rg = [list(range(P))]

    groups = [list(range(g, min(g + GBLK, NBLK))) for g in range(0, NBLK, GBLK)]
    calls_of_group = {}
    for c in calls:
        calls_of_group.setdefault((c[0], c[1]), []).append(c)

    rel = mybir.ActivationFunctionType.Relu
    cpy = mybir.ActivationFunctionType.Copy

    with tile.TileContext(nc) as tc:
        with ExitStack() as ctx:
            const = ctx.enter_context(tc.tile_pool(name="const", bufs=1))
            big = ctx.enter_context(tc.tile_pool(name="big", bufs=1))
            mpool = ctx.enter_context(tc.tile_pool(name="msg", bufs=8))
            opool = ctx.enter_context(tc.tile_pool(name="oh", bufs=8))
            wpool = ctx.enter_context(tc.tile_pool(name="work", bufs=6))
            xpool = ctx.enter_context(tc.tile_pool(name="xt", bufs=3))
            pacc = ctx.enter_context(tc.tile_pool(name="pacc", bufs=4, space="PSUM"))
            paux = ctx.enter_context(tc.tile_pool(name="paux", bufs=2, space="PSUM"))
            pden = ctx.enter_context(tc.tile_pool(name="pden", bufs=2, space="PSUM"))

            # ---- constants ----
            iota = const.tile([128, 128], DT.float32)
            nc.gpsimd.iota(iota[:], pattern=[[1, 128]], base=0, channel_multiplier=0,
                           allow_small_or_imprecise_dtypes=True)
            iota_h = const.tile([128, 128], BF16)
            nc.vector.tensor_copy(iota_h[:], iota[:])
            ident = const.tile([128, 128], DT.float32)
            nc.gpsimd.memset(ident[:], 0.0)
            nc.gpsimd.affine_select(ident[:], ident[:], pattern=[[-1, 128]],
                                    compare_op=mybir.AluOpType.not_equal, fill=1.0,
                                    base=0, channel_multiplier=1)
            ident_h = const.tile([128, 128], BF16)
            nc.vector.tensor_copy(ident_h[:], ident[:])

            idx_sb = const.tile([128, epad // 16], DT.int16)
            nc.sync.dma_start(idx_sb[:], idx_d[:])
            colrel_sb = const.tile([128, nchunk], BF16)
            nc.sync.dma_start(colrel_sb[:], colrel_d[:])
            dis_sb = const.tile([128, NBLK], DT.float32)
            nc.sync.dma_start(dis_sb[:], dis_d[:])
            dis2_sb = const.tile([128, NBLK], DT.float32)
            nc.sync.dma_start(dis2_sb[:], dis2_d[:])

            w0_sb = const.tile([8, 128], DT.float32)
            nc.sync.dma_start(w0_sb[:], w0_d[:])
            b0_sb = const.tile([128, 1], DT.float32)
            nc.sync.dma_start(b0_sb[:], b0_d[:])
            wtag_sb = []   # fp32, used for k=0 dense matmul
            wtagh_sb = []  # bf16, used for k>=1
            for i in range(n_g * nm1):
                if i % nm1 == 0:
                    t = const.tile([128, 128], DT.float32, tag=f"wtag{i}")
                    nc.sync.dma_start(t[:], wtag_d[i])
                    wtag_sb.append(t)
                    wtagh_sb.append(None)
                else:
                    t = const.tile([128, 128], BF16, tag=f"wtagh{i}")
                    nc.sync.dma_start(t[:], wtagh_d[i])
                    wtag_sb.append(None)
                    wtagh_sb.append(t)
            btag_sb = const.tile([128, n_g], DT.float32)
            nc.sync.dma_start(btag_sb[:], btag_d[:])
            wmlp_sb = []
            for i in range(n_m):
                t = const.tile([128, 128], DT.float32, tag=f"wmlp{i}")
                nc.sync.dma_start(t[:], wmlp_d[i])
                wmlp_sb.append(t)
            bmlp_sb = const.tile([128, n_m], DT.float32)
            nc.sync.dma_start(bmlp_sb[:], bmlp_d[:])
            w1_sb = const.tile([128, 1], DT.float32)
            nc.sync.dma_start(w1_sb[:], w1_d[:])
            b1_sb = const.tile([1, 1], DT.float32)
            nc.sync.dma_start(b1_sb[:], b1_d[:])

            hT = big.tile([128, NB], DT.float32)
            oT = big.tile([128, NB], DT.float32)

            # ---- lin0: hT = relu(W0^T xT + b0), emitted per slab inside the
            # layer-0 z0 walk so quarter-0's AllGather fires early ----
            lin0_done = set()

            def emit_lin0_slab(s):
                bb = 4 * s
                xt = xpool.tile([8, 512], DT.float32, tag="xt")
                nc.sync.dma_start(xt[:], xT_d[:, 128 * bb:128 * bb + 512])
                ph = pden.tile([128, 512], DT.float32, tag="ph")
                nc.tensor.matmul(ph[:], w0_sb[:], xt[:])
                nc.scalar.activation(hT[:, 128 * bb:128 * bb + 512], ph[:],
                                     rel, bias=b0_sb[:])

            par = 0
            qrr = [0]  # gather queue round-robin counter

            def quarter_of_block(b):
                return b // (NBLK // NQ)

            def emit_z0_block(b, zpar):
                # z0 = dis*h for block b -> zin[zpar]
                aux0 = paux.tile([128, 2, 128], DT.float32, tag="aux")
                nc.tensor.transpose(aux0[:, 0, :], hT[:, 128 * b:128 * (b + 1)], ident[:])
                zr = wpool.tile([128, 128], BF16, tag="zr")
                nc.scalar.activation(zr[:], aux0[:, 0, :], cpy, scale=dis_sb[:, b:b + 1])
                qq = quarter_of_block(b)
                lb = b - qq * (NBLK // NQ)
                nc.sync.dma_start(zin[zpar][qq][128 * lb:128 * (lb + 1), :], zr[:])

            def emit_ag(qq, zpar):
                nc.gpsimd.collective_compute(
                    "AllGather", mybir.AluOpType.bypass, replica_groups=rg,
                    ins=[zin[zpar][qq][:]], outs=[ztab[zpar][qq][:]])

            for g in range(n_g):
                # z0 blocks + quarter AGs (only blocks 0..97 hold real nodes)
                for qq in range(NQ):
                    for b in range(qq * 25, min(qq * 25 + 25, 98)):
                        s = b // 4
                        if g == 0 and s not in lin0_done:
                            lin0_done.add(s)
                            emit_lin0_slab(s)
                        emit_z0_block(b, par)
                    emit_ag(qq, par)
                if g == 0:
                    for s in range(NBLK // 4):
                        if s not in lin0_done:
                            lin0_done.add(s)
                            emit_lin0_slab(s)

                # out^T init: W[g,0]^T hT
                for bb in range(0, NBLK, 4):
                    po = pden.tile([128, 512], DT.float32, tag="ph")
                    nc.tensor.matmul(po[:], wtag_sb[g * nm1][:],
                                     hT[:, 128 * bb:128 * bb + 512])
                    nc.vector.tensor_copy(oT[:, 128 * bb:128 * bb + 512], po[:])

                for k in range(1, k_hops + 1):
                    nxt = par ^ 1
                    ag_fired = set()
                    for gi, blocks in enumerate(groups):
                        gcalls = [c for qq in range(NQ)
                                  for c in calls_of_group.get((gi, qq), [])]
                        if not gcalls:
                            continue  # pure-pad block group
                        acc = pacc.tile([128, GBLK, 128], DT.float32,
                                        name=f"acc_{g}_{k}_{gi}", tag="acc")
                        nc.scalar.memzero(acc[:])
                        for (gi_, qq, c_off, L, segs) in gcalls:
                            msg = mpool.tile([128, MAXL // 128, 128], BF16, tag="msg")
                            nc.gpsimd.dma_gather(
                                out_ap=msg[:, :L // 128, :],
                                in_ap=ztab[par][qq][:, :],
                                idxs_ap=idx_sb[:, c_off // 16:(c_off + L) // 16],
                                num_idxs=L, num_idxs_reg=L, elem_size=128,
                                queue_num=qrr[0] % NQUEUES)
                            qrr[0] += 1
                            # chunk -> block map for this call
                            cblocks = [b for (b, nch) in segs for _ in range(nch)]
                            nch = L // 128
                            c0 = c_off // 128
                            for q0 in range(0, nch, 4):
                                w = min(4, nch - q0)
                                oh = opool.tile([128, 4, 128], BF16, tag="oh")
                                in0 = iota_h[:].unsqueeze(1).broadcast_to([128, w, 128])
                                in1 = colrel_sb[:, c0 + q0:c0 + q0 + w].unsqueeze(2) \
                                    .broadcast_to([128, w, 128])
                                nc.vector.tensor_tensor(oh[:, :w, :], in0, in1,
                                                        op=mybir.AluOpType.is_equal)
                                for j in range(w):
                                    cj = q0 + j
                                    b = cblocks[cj]
                                    nc.tensor.matmul(
                                        acc[:, b - blocks[0], :], oh[:, j, :],
                                        msg[:, cj, :], start=False,
                                        stop=(c0 + cj == last_chunk[b]))
                        # finalize the group's blocks
                        for b in blocks:
                            if b not in chunks_of_block:
                                continue
                            jj = b - blocks[0]
                            if k < k_hops:
                                zr = wpool.tile([128, 128], BF16, tag="zr")
                                nc.scalar.activation(zr[:], acc[:, jj, :], cpy,
                                                     scale=dis2_sb[:, b:b + 1])
                                qq2 = quarter_of_block(b)
                                lb = b - qq2 * 25
                                nc.sync.dma_start(
                                    zin[nxt][qq2][128 * lb:128 * (lb + 1), :], zr[:])
                            xk = wpool.tile([128, 128], DT.float32, tag="xk")
                            nc.vector.tensor_scalar(xk[:], acc[:, jj, :],
                                                    dis_sb[:, b:b + 1], None,
                                                    op0=mybir.AluOpType.mult)
                            aux = paux.tile([128, 2, 128], DT.float32,
                                            name=f"aux_{g}_{k}_{b}", tag="aux")
                            nc.tensor.transpose(aux[:, 0, :], xk[:], ident[:])
                            xkT = wpool.tile([128, 128], BF16, tag="xkT")
                            nc.scalar.activation(xkT[:], aux[:, 0, :], cpy)
                            nc.tensor.matmul(aux[:, 1, :], wtagh_sb[g * nm1 + k][:],
                                             xkT[:])
                            nc.vector.tensor_add(oT[:, 128 * b:128 * (b + 1)],
                                                 oT[:, 128 * b:128 * (b + 1)],
                                                 aux[:, 1, :])
                        # fire AG for every quarter fully finalized by now
                        if k < k_hops:
                            for qq2 in range(NQ):
                                if qq2 not in ag_fired and blocks[-1] >= qq2 * 25 + 24:
                                    ag_fired.add(qq2)
                                    emit_ag(qq2, nxt)
                    if k < k_hops:
                        assert ag_fired == set(range(NQ))
                        par = nxt

                # layer end: h = relu(out + b_tag[g]), per 4-block slab so each
                # slab unblocks as soon as its hop-3 finalize lands
                for bb in range(0, NBLK, 4):
                    nc.scalar.activation(hT[:, 128 * bb:128 * bb + 512],
                                         oT[:, 128 * bb:128 * bb + 512],
                                         rel, bias=btag_sb[:, g:g + 1])

            # ---- MLP ----
            for m in range(n_m):
                src_t, dst_t = (hT, oT) if m % 2 == 0 else (oT, hT)
                for bb in range(0, NBLK, 4):
                    po = pden.tile([128, 512], DT.float32, tag="ph")
                    nc.tensor.matmul(po[:], wmlp_sb[m][:],
                                     src_t[:, 128 * bb:128 * bb + 512])
                    nc.scalar.activation(dst_t[:, 128 * bb:128 * bb + 512], po[:],
                                         rel, bias=bmlp_sb[:, m:m + 1])
            hT = oT if n_m % 2 == 1 else hT

            # ---- head ----
            ysb = big.tile([1, NB], DT.float32)
            for bb in range(0, NBLK, 4):
                py = pden.tile([1, 512], DT.float32, tag="ph")
                nc.tensor.matmul(py[:], w1_sb[:], hT[:, 128 * bb:128 * bb + 512])
                nc.scalar.activation(ysb[:, 128 * bb:128 * bb + 512], py[:],
                                     rel, bias=b1_sb[:])
                nc.sync.dma_start(y_d[:, 128 * bb:128 * bb + 512],
                                  ysb[:, 128 * bb:128 * bb + 512])

    nc.compile()
    return nc


def _setup(x, edge_index, W0, b0, W_tag, b_tag, W_mlp, b_mlp, W1, b1):
    x = np.asarray(x, np.float32)
    edge_index = np.asarray(edge_index)
    n_real = x.shape[0]
    n_g, nm1 = W_tag.shape[0], W_tag.shape[1]
    n_m = W_mlp.shape[0]

    ck = (n_real, edge_index.shape[1], int(edge_index[0, ::997].astype(np.int64).sum()),
          int(edge_index[1, ::997].astype(np.int64).sum()))
    if ck not in _cache:
        prep = _host_prep(edge_index, n_real)
        nc = _build(prep, n_g, nm1 - 1, n_m)
        _cache[ck] = (prep, nc)
    prep, nc = _cache[ck]

    npc = prep["npc"]
    xT = np.zeros((P, 8, NB), np.float32)
    xs = x.reshape(P, npc, -1)
    for c in range(P):
        xT[c, :xs.shape[2], :npc] = xs[c].T

    wtag = np.ascontiguousarray(W_tag.reshape(n_g * nm1, 128, 128), dtype=np.float32)
    in_maps = []
    for c in range(P):
        in_maps.append({
            "xT": xT[c], "idx": prep["idx16"][c], "colrel": prep["colrel"][c],
            "dis": prep["dis"][c], "dis2": prep["dis2"][c],
            "w0": np.vstack([np.asarray(W0, np.float32),
                             np.zeros((8 - W0.shape[0], 128), np.float32)]),
            "b0": np.asarray(b0, np.float32).reshape(128, 1),
            "wtag": wtag,
            "wtagh": wtag.astype(ml_dtypes.bfloat16),
            "btag": np.ascontiguousarray(np.asarray(b_tag, np.float32).T),
            "wmlp": np.asarray(W_mlp, np.float32),
            "bmlp": np.ascontiguousarray(np.asarray(b_mlp, np.float32).T),
            "w1": np.asarray(W1, np.float32),
            "b1": np.asarray(b1, np.float32).reshape(1, 1),
        })
    return nc, in_maps, npc, n_real


def kernel(**inputs):
    nc, in_maps, npc, n_real = _setup(**inputs)
    res = run_bass_kernel_spmd(nc, in_maps, list(range(P)))
    out = np.concatenate([res.results[c]["y"][0, :npc] for c in range(P)])
    return out.reshape(n_real, 1).astype(np.float32)


def run_traced(inputs):
    nc, in_maps, npc, n_real = _setup(**inputs)
    return run_bass_kernel_spmd(nc, in_maps, list(range(P)), trace=True)


# revision 7
# speedup vs baseline: 1.0463x; 1.0402x over previous
"""TAGConvNet (2x TAGConv K=3 + MLP) on 8 trn2 NeuronCores via Bass/Tile. v2.

Strategy (node-partition, 12800 padded rows/core = 100 blocks of 128):
- Message table z (= dis*x_k) replicated via per-quarter AllGathers in bf16.
  Quarter q = rows [3200q, 3200q+3200) of every core; quarter table is
  [8*3200=25600, 128] bf16 (int16-indexable).
- Per hop: for each group of 4 target blocks, gather padded per-(block,
  quarter) edge-source rows (bf16, 256B rows), build 4-wide one-hot tiles
  with a single broadcast tensor_tensor per quad, matmul-accumulate into a
  packed PSUM bank [128, 4, 128] (memzero + start=False).
- Finalize per block: zin(bf16) = dis2*acc -> DMA -> quarter AllGather
  (pipelined, fires when its 25 blocks are done); xk = dis*acc (bf16),
  transpose via PE, out^T += W_k^T xk^T.
- Dense layers (lin0 / W_g0 / MLP / head) run feature-major in fp32.
"""
import sys
from contextlib import ExitStack

import numpy as np

sys.path.insert(0, "/opt/trn_rl_repo")

import ml_dtypes  # noqa: E402
import concourse.bass as bass  # noqa: E402
import concourse.tile as tile  # noqa: E402
from concourse import bacc, mybir  # noqa: E402
from concourse.bass_utils import run_bass_kernel_spmd  # noqa: E402

P = 8                  # cores
NBLK = 100             # 128-node blocks per core
NB = NBLK * 128        # 12800 padded nodes per core
NTOT = P * NB          # 102400
NQ = 4                 # source quarters
QN = NB // NQ          # 3200 nodes per quarter per core
QSEG = P * QN          # 25600 rows per quarter gather table
GBLK = 4               # blocks per group (packed psum bank)
MAXL = 1024            # max idxs per dma_gather call (2048 wedges the runtime)
DT = mybir.dt
BF16 = DT.bfloat16
NQUEUES = 1            # SWDGE queues to round-robin gathers over

_cache = {}


def _host_prep(edge_index, n_real):
    npc = n_real // P  # 12500 real nodes per core
    row, col = edge_index[0].astype(np.int64), edge_index[1].astype(np.int64)

    deg = np.bincount(col, minlength=n_real)
    dis = np.where(deg > 0, 1.0 / np.sqrt(np.maximum(deg, 1.0)), 0.0).astype(np.float32)

    core_s, loc_s = row // npc, row % npc
    core_t, loc_t = col // npc, col % npc
    blk = loc_t >> 7                      # target block 0..97
    q = loc_s // QN                       # source quarter 0..3
    srow = core_s * QN + (loc_s - q * QN)  # row in quarter table

    cnt = np.zeros((P, NBLK, NQ), np.int64)
    np.add.at(cnt, (core_t, blk, q), 1)
    pbs = (128 * np.ceil(cnt.max(axis=0) / 128.0)).astype(np.int64)  # [NBLK, NQ]

    groups = [list(range(g, min(g + GBLK, NBLK))) for g in range(0, NBLK, GBLK)]
    off = np.zeros((NBLK, NQ), np.int64)
    pos = 0
    calls = []  # (group_idx, q, stream_off, L, [(block, nchunks), ...])
    for gi, blocks in enumerate(groups):
        for qq in range(NQ):
            cur = None
            for b in blocks:
                n = int(pbs[b, qq])
                if n == 0:
                    continue
                off[b, qq] = pos
                if cur is not None and cur[3] + n <= MAXL:
                    cur[3] += n
                    cur[4].append((b, n // 128))
                else:
                    if cur is not None:
                        calls.append(tuple(cur))
                    cur = [gi, qq, pos, n, [(b, n // 128)]]
                pos += n
            if cur is not None:
                calls.append(tuple(cur))
    epad = pos

    # slot assignment
    key = (core_t * NBLK + blk) * NQ + q
    order = np.argsort(key, kind="stable")
    key_s = key[order]
    first = np.searchsorted(key_s, key_s)
    rank = np.arange(len(key_s)) - first
    dst = off[blk[order], q[order]] + rank

    gidx = np.zeros((P, epad), np.int16)
    colrel = np.full((P, epad), -1.0, np.float32)
    gidx[core_t[order], dst] = srow[order].astype(np.int16)
    colrel[core_t[order], dst] = (loc_t[order] - blk[order] * 128).astype(np.float32)

    dis_g = np.zeros((P, NB), np.float32)
    dis_g[:, :npc] = dis.reshape(P, npc)

    idx16 = np.tile(gidx.reshape(P, epad // 16, 16).transpose(0, 2, 1), (1, 8, 1)).copy()
    colrel128 = np.ascontiguousarray(
        colrel.reshape(P, epad // 128, 128).transpose(0, 2, 1)
    ).astype(ml_dtypes.bfloat16)  # [P, 128, NCHUNK]
    dis_blk = np.ascontiguousarray(
        dis_g.reshape(P, NBLK, 128).transpose(0, 2, 1))  # [P, 128, NBLK]
    return dict(epad=epad, calls=calls, idx16=idx16, colrel=colrel128,
                dis=dis_blk, dis2=dis_blk * dis_blk, npc=npc)


def _build(prep, n_g, k_hops, n_m):
    epad = prep["epad"]
    calls = prep["calls"]
    nm1 = k_hops + 1
    nchunk = epad // 128

    # last chunk (global chunk id) per block, for matmul stop flags
    last_chunk = {}
    chunks_of_block = {}
    for (gi, qq, c_off, L, segs) in calls:
        j = c_off // 128
        for (b, nch) in segs:
            for t in range(nch):
                last_chunk[b] = j
                chunks_of_block.setdefault(b, []).append(j)
                j += 1

    nc = bacc.Bacc("TRN2", target_bir_lowering=False, debug=False,
                   num_devices=P, num_swdge_queues=4)

    xT_d = nc.dram_tensor("xT", [8, NB], DT.float32, kind="ExternalInput")
    idx_d = nc.dram_tensor("idx", [128, epad // 16], DT.int16, kind="ExternalInput")
    colrel_d = nc.dram_tensor("colrel", [128, nchunk], BF16, kind="ExternalInput")
    dis_d = nc.dram_tensor("dis", [128, NBLK], DT.float32, kind="ExternalInput")
    dis2_d = nc.dram_tensor("dis2", [128, NBLK], DT.float32, kind="ExternalInput")
    w0_d = nc.dram_tensor("w0", [8, 128], DT.float32, kind="ExternalInput")
    b0_d = nc.dram_tensor("b0", [128, 1], DT.float32, kind="ExternalInput")
    wtag_d = nc.dram_tensor("wtag", [n_g * nm1, 128, 128], DT.float32, kind="ExternalInput")
    wtagh_d = nc.dram_tensor("wtagh", [n_g * nm1, 128, 128], BF16, kind="ExternalInput")
    btag_d = nc.dram_tensor("btag", [128, n_g], DT.float32, kind="ExternalInput")
    wmlp_d = nc.dram_tensor("wmlp", [n_m, 128, 128], DT.float32, kind="ExternalInput")
    bmlp_d = nc.dram_tensor("bmlp", [128, n_m], DT.float32, kind="ExternalInput")
    w1_d = nc.dram_tensor("w1", [128, 1], DT.float32, kind="ExternalInput")
    b1_d = nc.dram_tensor("b1", [1, 1], DT.float32, kind="ExternalInput")
    y_d = nc.dram_tensor("y", [1, NB], DT.float32, kind="ExternalOutput")

    zin = [[nc.dram_tensor(f"zin{par}_{qq}", [QN, 128], BF16)
            for qq in range(NQ)] for par in range(2)]
    ztab = [[nc.dram_tensor(f"ztab{par}_{qq}", [QSEG, 128], BF16, addr_space="Shared")
             for qq in range(NQ)] for par in range(2)]
    rg = [list(range(P))]

    groups = [list(range(g, min(g + GBLK, NBLK))) for g in range(0, NBLK, GBLK)]
    calls_of_group = {}
    for c in calls:
        calls_of_group.setdefault((c[0], c[1]), []).append(c)

    rel = mybir.ActivationFunctionType.Relu
    cpy = mybir.ActivationFunctionType.Copy

    with tile.TileContext(nc) as tc:
        with ExitStack() as ctx:
            const = ctx.enter_context(tc.tile_pool(name="const", bufs=1))
            big = ctx.enter_context(tc.tile_pool(name="big", bufs=1))
            mpool = ctx.enter_context(tc.tile_pool(name="msg", bufs=8))
            opool = ctx.enter_context(tc.tile_pool(name="oh", bufs=8))
            wpool = ctx.enter_context(tc.tile_pool(name="work", bufs=6))
            xpool = ctx.enter_context(tc.tile_pool(name="xt", bufs=3))
            pacc = ctx.enter_context(tc.tile_pool(name="pacc", bufs=4, space="PSUM"))
            paux = ctx.enter_context(tc.tile_pool(name="paux", bufs=2, space="PSUM"))
            pden = ctx.enter_context(tc.tile_pool(name="pden", bufs=2, space="PSUM"))

            # ---- constants ----
            iota = const.tile([128, 128], DT.float32)
            nc.gpsimd.iota(iota[:], pattern=[[1, 128]], base=0, channel_multiplier=0,
                           allow_small_or_imprecise_dtypes=True)
            iota_h = const.tile([128, 128], BF16)
            nc.vector.tensor_copy(iota_h[:], iota[:])
            ident = const.tile([128, 128], DT.float32)
            nc.gpsimd.memset(ident[:], 0.0)
            nc.gpsimd.affine_select(ident[:], ident[:], pattern=[[-1, 128]],
                                    compare_op=mybir.AluOpType.not_equal, fill=1.0,
                                    base=0, channel_multiplier=1)
            ident_h = const.tile([128, 128], BF16)
            nc.vector.tensor_copy(ident_h[:], ident[:])

            idx_sb = const.tile([128, epad // 16], DT.int16)
            nc.sync.dma_start(idx_sb[:], idx_d[:])
            colrel_sb = const.tile([128, nchunk], BF16)
            nc.sync.dma_start(colrel_sb[:], colrel_d[:])
            dis_sb = const.tile([128, NBLK], DT.float32)
            nc.sync.dma_start(dis_sb[:], dis_d[:])
            dis2_sb = const.tile([128, NBLK], DT.float32)
            nc.sync.dma_start(dis2_sb[:], dis2_d[:])

            w0_sb = const.tile([8, 128], DT.float32)
            nc.sync.dma_start(w0_sb[:], w0_d[:])
            b0_sb = const.tile([128, 1], DT.float32)
            nc.sync.dma_start(b0_sb[:], b0_d[:])
            wtag_sb = []   # fp32, used for k=0 dense matmul
            wtagh_sb = []  # bf16, used for k>=1
            for i in range(n_g * nm1):
                if i % nm1 == 0:
                    t = const.tile([128, 128], DT.float32, tag=f"wtag{i}")
                    nc.sync.dma_start(t[:], wtag_d[i])
                    wtag_sb.append(t)
                    wtagh_sb.append(None)
                else:
                    t = const.tile([128, 128], BF16, tag=f"wtagh{i}")
                    nc.sync.dma_start(t[:], wtagh_d[i])
                    wtag_sb.append(None)
                    wtagh_sb.append(t)
            btag_sb = const.tile([128, n_g], DT.float32)
            nc.sync.dma_start(btag_sb[:], btag_d[:])
            wmlp_sb = []
            for i in range(n_m):
                t = const.tile([128, 128], DT.float32, tag=f"wmlp{i}")
                nc.sync.dma_start(t[:], wmlp_d[i])
                wmlp_sb.append(t)
            bmlp_sb = const.tile([128, n_m], DT.float32)
            nc.sync.dma_start(bmlp_sb[:], bmlp_d[:])
            w1_sb = const.tile([128, 1], DT.float32)
            nc.sync.dma_start(w1_sb[:], w1_d[:])
            b1_sb = const.tile([1, 1], DT.float32)
            nc.sync.dma_start(b1_sb[:], b1_d[:])

            hT = big.tile([128, NB], DT.float32)
            oT = big.tile([128, NB], DT.float32)

            # ---- lin0: hT = relu(W0^T xT + b0), emitted per slab inside the
            # layer-0 z0 walk so quarter-0's AllGather fires early ----
            lin0_done = set()

            def emit_lin0_slab(s):
                bb = 4 * s
                xt = xpool.tile([8, 512], DT.float32, tag="xt")
                nc.sync.dma_start(xt[:], xT_d[:, 128 * bb:128 * bb + 512])
                ph = pden.tile([128, 512], DT.float32, tag="ph")
                nc.tensor.matmul(ph[:], w0_sb[:], xt[:])
                nc.scalar.activation(hT[:, 128 * bb:128 * bb + 512], ph[:],
                                     rel, bias=b0_sb[:])

            par = 0
            qrr = [0]  # gather queue round-robin counter

            def quarter_of_block(b):
                return b // (NBLK // NQ)

            def emit_z0_block(b, zpar):
                # z0 = dis*h for block b -> zin[zpar]
                aux0 = paux.tile([128, 2, 128], DT.float32, tag="aux")
                nc.tensor.transpose(aux0[:, 0, :], hT[:, 128 * b:128 * (b + 1)], ident[:])
                zr = wpool.tile([128, 128], BF16, tag="zr")
                nc.scalar.activation(zr[:], aux0[:, 0, :], cpy, scale=dis_sb[:, b:b + 1])
                qq = quarter_of_block(b)
                lb = b - qq * (NBLK // NQ)
                nc.sync.dma_start(zin[zpar][qq][128 * lb:128 * (lb + 1), :], zr[:])

            def emit_ag(qq, zpar):
                nc.gpsimd.collective_compute(
                    "AllGather", mybir.AluOpType.bypass, replica_groups=rg,
                    ins=[zin[zpar][qq][:]], outs=[ztab[zpar][qq][:]])

            for g in range(n_g):
                # z0 blocks + quarter AGs; for g > 0 these were emitted early,
                # interleaved with the previous layer's hop-3 (see below)
                if g == 0:
                    for qq in range(NQ):
                        for b in range(qq * 25, min(qq * 25 + 25, 98)):
                            s = b // 4
                            if s not in lin0_done:
                                lin0_done.add(s)
                                emit_lin0_slab(s)
                            emit_z0_block(b, par)
                        emit_ag(qq, par)
                    for s in range(NBLK // 4):
                        if s not in lin0_done:
                            lin0_done.add(s)
                            emit_lin0_slab(s)

                # out^T init: W[g,0]^T hT
                for bb in range(0, NBLK, 4):
                    po = pden.tile([128, 512], DT.float32, tag="ph")
                    nc.tensor.matmul(po[:], wtag_sb[g * nm1][:],
                                     hT[:, 128 * bb:128 * bb + 512])
                    nc.vector.tensor_copy(oT[:, 128 * bb:128 * bb + 512], po[:])

                for k in range(1, k_hops + 1):
                    nxt = par ^ 1
                    ag_fired = set()
                    z0_fired = set()
                    for gi, blocks in enumerate(groups):
                        gcalls = [c for qq in range(NQ)
                                  for c in calls_of_group.get((gi, qq), [])]
                        if not gcalls:
                            continue  # pure-pad block group
                        acc = pacc.tile([128, GBLK, 128], DT.float32,
                                        name=f"acc_{g}_{k}_{gi}", tag="acc")
                        nc.scalar.memzero(acc[:])
                        for (gi_, qq, c_off, L, segs) in gcalls:
                            msg = mpool.tile([128, MAXL // 128, 128], BF16, tag="msg")
                            nc.gpsimd.dma_gather(
                                out_ap=msg[:, :L // 128, :],
                                in_ap=ztab[par][qq][:, :],
                                idxs_ap=idx_sb[:, c_off // 16:(c_off + L) // 16],
                                num_idxs=L, num_idxs_reg=L, elem_size=128,
                                queue_num=qrr[0] % NQUEUES)
                            qrr[0] += 1
                            # chunk -> block map for this call
                            cblocks = [b for (b, nch) in segs for _ in range(nch)]
                            nch = L // 128
                            c0 = c_off // 128
                            for q0 in range(0, nch, 4):
                                w = min(4, nch - q0)
                                oh = opool.tile([128, 4, 128], BF16, tag="oh")
                                in0 = iota_h[:].unsqueeze(1).broadcast_to([128, w, 128])
                                in1 = colrel_sb[:, c0 + q0:c0 + q0 + w].unsqueeze(2) \
                                    .broadcast_to([128, w, 128])
                                nc.vector.tensor_tensor(oh[:, :w, :], in0, in1,
                                                        op=mybir.AluOpType.is_equal)
                                for j in range(w):
                                    cj = q0 + j
                                    b = cblocks[cj]
                                    nc.tensor.matmul(
                                        acc[:, b - blocks[0], :], oh[:, j, :],
                                        msg[:, cj, :], start=False,
                                        stop=(c0 + cj == last_chunk[b]))
                        # finalize the group's blocks
                        for b in blocks:
                            if b not in chunks_of_block:
                                continue
                            jj = b - blocks[0]
                            if k < k_hops:
                                zr = wpool.tile([128, 128], BF16, tag="zr")
                                nc.scalar.activation(zr[:], acc[:, jj, :], cpy,
                                                     scale=dis2_sb[:, b:b + 1])
                                qq2 = quarter_of_block(b)
                                lb = b - qq2 * 25
                                nc.sync.dma_start(
                                    zin[nxt][qq2][128 * lb:128 * (lb + 1), :], zr[:])
                            xk = wpool.tile([128, 128], DT.float32, tag="xk")
                            nc.vector.tensor_scalar(xk[:], acc[:, jj, :],
                                                    dis_sb[:, b:b + 1], None,
                                                    op0=mybir.AluOpType.mult)
                            aux = paux.tile([128, 2, 128], DT.float32,
                                            name=f"aux_{g}_{k}_{b}", tag="aux")
                            nc.tensor.transpose(aux[:, 0, :], xk[:], ident[:])
                            xkT = wpool.tile([128, 128], BF16, tag="xkT")
                            nc.scalar.activation(xkT[:], aux[:, 0, :], cpy)
                            nc.tensor.matmul(aux[:, 1, :], wtagh_sb[g * nm1 + k][:],
                                             xkT[:])
                            nc.vector.tensor_add(oT[:, 128 * b:128 * (b + 1)],
                                                 oT[:, 128 * b:128 * (b + 1)],
                                                 aux[:, 1, :])
                        # fire AG for every quarter fully finalized by now
                        if k < k_hops:
                            for qq2 in range(NQ):
                                if qq2 not in ag_fired and blocks[-1] >= qq2 * 25 + 24:
                                    ag_fired.add(qq2)
                                    emit_ag(qq2, nxt)
                        else:
                            # last hop: relu this slab now (GBLK == slab width) and
                            # emit next layer's z0 on the opposite parity so its
                            # AllGathers overlap the rest of this hop
                            bb = blocks[0]
                            nc.scalar.activation(hT[:, 128 * bb:128 * bb + 512],
                                                 oT[:, 128 * bb:128 * bb + 512],
                                                 rel, bias=btag_sb[:, g:g + 1])
                            if g + 1 < n_g:
                                for b in blocks:
                                    if b < 98:
                                        emit_z0_block(b, par ^ 1)
                                for qq2 in range(NQ):
                                    if qq2 not in z0_fired and \
                                       blocks[-1] >= qq2 * 25 + 24:
                                        z0_fired.add(qq2)
                                        emit_ag(qq2, par ^ 1)
                    if k < k_hops:
                        assert ag_fired == set(range(NQ))
                        par = nxt
                if g + 1 < n_g:
                    par ^= 1



            # ---- MLP ----
            for m in range(n_m):
                src_t, dst_t = (hT, oT) if m % 2 == 0 else (oT, hT)
                for bb in range(0, NBLK, 4):
                    po = pden.tile([128, 512], DT.float32, tag="ph")
                    nc.tensor.matmul(po[:], wmlp_sb[m][:],
                                     src_t[:, 128 * bb:128 * bb + 512])
                    nc.scalar.activation(dst_t[:, 128 * bb:128 * bb + 512], po[:],
                                         rel, bias=bmlp_sb[:, m:m + 1])
            hT = oT if n_m % 2 == 1 else hT

            # ---- head ----
            ysb = big.tile([1, NB], DT.float32)
            for bb in range(0, NBLK, 4):
                py = pden.tile([1, 512], DT.float32, tag="ph")
                nc.tensor.matmul(py[:], w1_sb[:], hT[:, 128 * bb:128 * bb + 512])
                nc.scalar.activation(ysb[:, 128 * bb:128 * bb + 512], py[:],
                                     rel, bias=b1_sb[:])
                nc.sync.dma_start(y_d[:, 128 * bb:128 * bb + 512],
                                  ysb[:, 128 * bb:128 * bb + 512])

    nc.compile()
    return nc


def _setup(x, edge_index, W0, b0, W_tag, b_tag, W_mlp, b_mlp, W1, b1):
    x = np.asarray(x, np.float32)
    edge_index = np.asarray(edge_index)
    n_real = x.shape[0]
    n_g, nm1 = W_tag.shape[0], W_tag.shape[1]
    n_m = W_mlp.shape[0]

    ck = (n_real, edge_index.shape[1], int(edge_index[0, ::997].astype(np.int64).sum()),
          int(edge_index[1, ::997].astype(np.int64).sum()))
    if ck not in _cache:
        prep = _host_prep(edge_index, n_real)
        nc = _build(prep, n_g, nm1 - 1, n_m)
        _cache[ck] = (prep, nc)
    prep, nc = _cache[ck]

    npc = prep["npc"]
    xT = np.zeros((P, 8, NB), np.float32)
    xs = x.reshape(P, npc, -1)
    for c in range(P):
        xT[c, :xs.shape[2], :npc] = xs[c].T

    wtag = np.ascontiguousarray(W_tag.reshape(n_g * nm1, 128, 128), dtype=np.float32)
    in_maps = []
    for c in range(P):
        in_maps.append({
            "xT": xT[c], "idx": prep["idx16"][c], "colrel": prep["colrel"][c],
            "dis": prep["dis"][c], "dis2": prep["dis2"][c],
            "w0": np.vstack([np.asarray(W0, np.float32),
                             np.zeros((8 - W0.shape[0], 128), np.float32)]),
            "b0": np.asarray(b0, np.float32).reshape(128, 1),
            "wtag": wtag,
            "wtagh": wtag.astype(ml_dtypes.bfloat16),
            "btag": np.ascontiguousarray(np.asarray(b_tag, np.float32).T),
            "wmlp": np.asarray(W_mlp, np.float32),
            "bmlp": np.ascontiguousarray(np.asarray(b_mlp, np.float32).T),
            "w1": np.asarray(W1, np.float32),
            "b1": np.asarray(b1, np.float32).reshape(1, 1),
        })
    return nc, in_maps, npc, n_real


def kernel(**inputs):
    nc, in_maps, npc, n_real = _setup(**inputs)
    res = run_bass_kernel_spmd(nc, in_maps, list(range(P)))
    out = np.concatenate([res.results[c]["y"][0, :npc] for c in range(P)])
    return out.reshape(n_real, 1).astype(np.float32)


def run_traced(inputs):
    nc, in_maps, npc, n_real = _setup(**inputs)
    return run_bass_kernel_spmd(nc, in_maps, list(range(P)), trace=True)


# revision 8
# speedup vs baseline: 1.0613x; 1.0143x over previous
"""TAGConvNet (2x TAGConv K=3 + MLP) on 8 trn2 NeuronCores via Bass/Tile. v2.

Strategy (node-partition, 12800 padded rows/core = 100 blocks of 128):
- Message table z (= dis*x_k) replicated via per-quarter AllGathers in bf16.
  Quarter q = rows [3200q, 3200q+3200) of every core; quarter table is
  [8*3200=25600, 128] bf16 (int16-indexable).
- Per hop: for each group of 4 target blocks, gather padded per-(block,
  quarter) edge-source rows (bf16, 256B rows), build 4-wide one-hot tiles
  with a single broadcast tensor_tensor per quad, matmul-accumulate into a
  packed PSUM bank [128, 4, 128] (memzero + start=False).
- Finalize per block: zin(bf16) = dis2*acc -> DMA -> quarter AllGather
  (pipelined, fires when its 25 blocks are done); xk = dis*acc (bf16),
  transpose via PE, out^T += W_k^T xk^T.
- Dense layers (lin0 / W_g0 / MLP / head) run feature-major in fp32.
"""
import sys
from contextlib import ExitStack

import numpy as np

sys.path.insert(0, "/opt/trn_rl_repo")

import ml_dtypes  # noqa: E402
import concourse.bass as bass  # noqa: E402
import concourse.tile as tile  # noqa: E402
from concourse import bacc, mybir  # noqa: E402
from concourse.bass_utils import run_bass_kernel_spmd  # noqa: E402

P = 8                  # cores
NBLK = 100             # 128-node blocks per core
NB = NBLK * 128        # 12800 padded nodes per core
NTOT = P * NB          # 102400
NQ = 4                 # source quarters
QN = NB // NQ          # 3200 nodes per quarter per core
QSEG = P * QN          # 25600 rows per quarter gather table
GBLK = 4               # blocks per group (packed psum bank)
MAXL = 1024            # max idxs per dma_gather call (2048 wedges the runtime)
DT = mybir.dt
BF16 = DT.bfloat16
NQUEUES = 1            # SWDGE queues to round-robin gathers over

_cache = {}


def _host_prep(edge_index, n_real):
    npc = n_real // P  # 12500 real nodes per core
    row, col = edge_index[0].astype(np.int64), edge_index[1].astype(np.int64)

    deg = np.bincount(col, minlength=n_real)
    dis = np.where(deg > 0, 1.0 / np.sqrt(np.maximum(deg, 1.0)), 0.0).astype(np.float32)

    core_s, loc_s = row // npc, row % npc
    core_t, loc_t = col // npc, col % npc
    blk = loc_t >> 7                      # target block 0..97
    q = loc_s // QN                       # source quarter 0..3
    srow = core_s * QN + (loc_s - q * QN)  # row in quarter table

    cnt = np.zeros((P, NBLK, NQ), np.int64)
    np.add.at(cnt, (core_t, blk, q), 1)
    pbs = (128 * np.ceil(cnt.max(axis=0) / 128.0)).astype(np.int64)  # [NBLK, NQ]

    groups = [list(range(g, min(g + GBLK, NBLK))) for g in range(0, NBLK, GBLK)]
    off = np.zeros((NBLK, NQ), np.int64)
    pos = 0
    calls = []  # (group_idx, q, stream_off, L, [(block, nchunks), ...])
    for gi, blocks in enumerate(groups):
        for qq in range(NQ):
            cur = None
            for b in blocks:
                n = int(pbs[b, qq])
                if n == 0:
                    continue
                off[b, qq] = pos
                if cur is not None and cur[3] + n <= MAXL:
                    cur[3] += n
                    cur[4].append((b, n // 128))
                else:
                    if cur is not None:
                        calls.append(tuple(cur))
                    cur = [gi, qq, pos, n, [(b, n // 128)]]
                pos += n
            if cur is not None:
                calls.append(tuple(cur))
    epad = pos

    # slot assignment
    key = (core_t * NBLK + blk) * NQ + q
    order = np.argsort(key, kind="stable")
    key_s = key[order]
    first = np.searchsorted(key_s, key_s)
    rank = np.arange(len(key_s)) - first
    dst = off[blk[order], q[order]] + rank

    gidx = np.zeros((P, epad), np.int16)
    colrel = np.full((P, epad), -1.0, np.float32)
    gidx[core_t[order], dst] = srow[order].astype(np.int16)
    colrel[core_t[order], dst] = (loc_t[order] - blk[order] * 128).astype(np.float32)

    dis_g = np.zeros((P, NB), np.float32)
    dis_g[:, :npc] = dis.reshape(P, npc)

    idx16 = np.tile(gidx.reshape(P, epad // 16, 16).transpose(0, 2, 1), (1, 8, 1)).copy()
    colrel128 = np.ascontiguousarray(
        colrel.reshape(P, epad // 128, 128).transpose(0, 2, 1)
    ).astype(ml_dtypes.bfloat16)  # [P, 128, NCHUNK]
    dis_blk = np.ascontiguousarray(
        dis_g.reshape(P, NBLK, 128).transpose(0, 2, 1))  # [P, 128, NBLK]
    return dict(epad=epad, calls=calls, idx16=idx16, colrel=colrel128,
                dis=dis_blk, dis2=dis_blk * dis_blk, npc=npc)


def _build(prep, n_g, k_hops, n_m):
    epad = prep["epad"]
    calls = prep["calls"]
    nm1 = k_hops + 1
    nchunk = epad // 128

    # last chunk (global chunk id) per block, for matmul stop flags
    last_chunk = {}
    chunks_of_block = {}
    for (gi, qq, c_off, L, segs) in calls:
        j = c_off // 128
        for (b, nch) in segs:
            for t in range(nch):
                last_chunk[b] = j
                chunks_of_block.setdefault(b, []).append(j)
                j += 1

    nc = bacc.Bacc("TRN2", target_bir_lowering=False, debug=False,
                   num_devices=P, num_swdge_queues=4)

    xT_d = nc.dram_tensor("xT", [8, NB], DT.float32, kind="ExternalInput")
    idx_d = nc.dram_tensor("idx", [128, epad // 16], DT.int16, kind="ExternalInput")
    colrel_d = nc.dram_tensor("colrel", [128, nchunk], BF16, kind="ExternalInput")
    dis_d = nc.dram_tensor("dis", [128, NBLK], DT.float32, kind="ExternalInput")
    dis2_d = nc.dram_tensor("dis2", [128, NBLK], DT.float32, kind="ExternalInput")
    w0_d = nc.dram_tensor("w0", [8, 128], DT.float32, kind="ExternalInput")
    b0_d = nc.dram_tensor("b0", [128, 1], DT.float32, kind="ExternalInput")
    wtag_d = nc.dram_tensor("wtag", [n_g * nm1, 128, 128], DT.float32, kind="ExternalInput")
    wtagh_d = nc.dram_tensor("wtagh", [n_g * nm1, 128, 128], BF16, kind="ExternalInput")
    btag_d = nc.dram_tensor("btag", [128, n_g], DT.float32, kind="ExternalInput")
    wmlp_d = nc.dram_tensor("wmlp", [n_m, 128, 128], DT.float32, kind="ExternalInput")
    bmlp_d = nc.dram_tensor("bmlp", [128, n_m], DT.float32, kind="ExternalInput")
    w1_d = nc.dram_tensor("w1", [128, 1], DT.float32, kind="ExternalInput")
    b1_d = nc.dram_tensor("b1", [1, 1], DT.float32, kind="ExternalInput")
    y_d = nc.dram_tensor("y", [1, NB], DT.float32, kind="ExternalOutput")

    zin = [[nc.dram_tensor(f"zin{par}_{qq}", [QN, 128], BF16)
            for qq in range(NQ)] for par in range(2)]
    ztab = [[nc.dram_tensor(f"ztab{par}_{qq}", [QSEG, 128], BF16, addr_space="Shared")
             for qq in range(NQ)] for par in range(2)]
    rg = [list(range(P))]

    groups = [list(range(g, min(g + GBLK, NBLK))) for g in range(0, NBLK, GBLK)]
    calls_of_group = {}
    for c in calls:
        calls_of_group.setdefault((c[0], c[1]), []).append(c)

    rel = mybir.ActivationFunctionType.Relu
    cpy = mybir.ActivationFunctionType.Copy

    with tile.TileContext(nc) as tc:
        with ExitStack() as ctx:
            const = ctx.enter_context(tc.tile_pool(name="const", bufs=1))
            big = ctx.enter_context(tc.tile_pool(name="big", bufs=1))
            mpool = ctx.enter_context(tc.tile_pool(name="msg", bufs=8))
            opool = ctx.enter_context(tc.tile_pool(name="oh", bufs=8))
            wpool = ctx.enter_context(tc.tile_pool(name="work", bufs=6))
            xpool = ctx.enter_context(tc.tile_pool(name="xt", bufs=3))
            pacc = ctx.enter_context(tc.tile_pool(name="pacc", bufs=4, space="PSUM"))
            paux = ctx.enter_context(tc.tile_pool(name="paux", bufs=2, space="PSUM"))
            pden = ctx.enter_context(tc.tile_pool(name="pden", bufs=2, space="PSUM"))

            # ---- constants ----
            iota = const.tile([128, 128], DT.float32)
            nc.gpsimd.iota(iota[:], pattern=[[1, 128]], base=0, channel_multiplier=0,
                           allow_small_or_imprecise_dtypes=True)
            iota_h = const.tile([128, 128], BF16)
            nc.vector.tensor_copy(iota_h[:], iota[:])
            ident = const.tile([128, 128], DT.float32)
            nc.gpsimd.memset(ident[:], 0.0)
            nc.gpsimd.affine_select(ident[:], ident[:], pattern=[[-1, 128]],
                                    compare_op=mybir.AluOpType.not_equal, fill=1.0,
                                    base=0, channel_multiplier=1)
            ident_h = const.tile([128, 128], BF16)
            nc.vector.tensor_copy(ident_h[:], ident[:])

            idx_sb = const.tile([128, epad // 16], DT.int16)
            nc.sync.dma_start(idx_sb[:], idx_d[:])
            colrel_sb = const.tile([128, nchunk], BF16)
            nc.sync.dma_start(colrel_sb[:], colrel_d[:])
            dis_sb = const.tile([128, NBLK], DT.float32)
            nc.sync.dma_start(dis_sb[:], dis_d[:])
            dis2_sb = const.tile([128, NBLK], DT.float32)
            nc.sync.dma_start(dis2_sb[:], dis2_d[:])

            w0_sb = const.tile([8, 128], DT.float32)
            nc.sync.dma_start(w0_sb[:], w0_d[:])
            b0_sb = const.tile([128, 1], DT.float32)
            nc.sync.dma_start(b0_sb[:], b0_d[:])
            wtag_sb = []   # fp32, used for k=0 dense matmul
            wtagh_sb = []  # bf16, used for k>=1
            for i in range(n_g * nm1):
                if i % nm1 == 0:
                    t = const.tile([128, 128], DT.float32, tag=f"wtag{i}")
                    nc.sync.dma_start(t[:], wtag_d[i])
                    wtag_sb.append(t)
                    wtagh_sb.append(None)
                else:
                    t = const.tile([128, 128], BF16, tag=f"wtagh{i}")
                    nc.sync.dma_start(t[:], wtagh_d[i])
                    wtag_sb.append(None)
                    wtagh_sb.append(t)
            btag_sb = const.tile([128, n_g], DT.float32)
            nc.sync.dma_start(btag_sb[:], btag_d[:])
            wmlp_sb = []
            for i in range(n_m):
                t = const.tile([128, 128], DT.float32, tag=f"wmlp{i}")
                nc.sync.dma_start(t[:], wmlp_d[i])
                wmlp_sb.append(t)
            bmlp_sb = const.tile([128, n_m], DT.float32)
            nc.sync.dma_start(bmlp_sb[:], bmlp_d[:])
            w1_sb = const.tile([128, 1], DT.float32)
            nc.sync.dma_start(w1_sb[:], w1_d[:])
            b1_sb = const.tile([1, 1], DT.float32)
            nc.sync.dma_start(b1_sb[:], b1_d[:])

            hT = big.tile([128, NB], DT.float32)
            oT = big.tile([128, NB], DT.float32)

            # ---- lin0: hT = relu(W0^T xT + b0), emitted per slab inside the
            # layer-0 z0 walk so quarter-0's AllGather fires early ----
            lin0_done = set()

            def emit_lin0_slab(s):
                bb = 4 * s
                xt = xpool.tile([8, 512], DT.float32, tag="xt")
                nc.sync.dma_start(xt[:], xT_d[:, 128 * bb:128 * bb + 512])
                ph = pden.tile([128, 512], DT.float32, tag="ph")
                nc.tensor.matmul(ph[:], w0_sb[:], xt[:])
                nc.scalar.activation(hT[:, 128 * bb:128 * bb + 512], ph[:],
                                     rel, bias=b0_sb[:])

            par = 0
            qrr = [0]  # gather queue round-robin counter

            def quarter_of_block(b):
                return b // (NBLK // NQ)

            def emit_z0_block(b, zpar):
                # z0 = dis*h for block b -> zin[zpar]
                aux0 = paux.tile([128, 2, 128], DT.float32, tag="aux")
                nc.tensor.transpose(aux0[:, 0, :], hT[:, 128 * b:128 * (b + 1)], ident[:])
                zr = wpool.tile([128, 128], BF16, tag="zr")
                nc.scalar.activation(zr[:], aux0[:, 0, :], cpy, scale=dis_sb[:, b:b + 1])
                qq = quarter_of_block(b)
                lb = b - qq * (NBLK // NQ)
                nc.sync.dma_start(zin[zpar][qq][128 * lb:128 * (lb + 1), :], zr[:])

            def emit_ag(qq, zpar):
                nc.gpsimd.collective_compute(
                    "AllGather", mybir.AluOpType.bypass, replica_groups=rg,
                    ins=[zin[zpar][qq][:]], outs=[ztab[zpar][qq][:]])

            for g in range(n_g):
                # z0 blocks + quarter AGs; for g > 0 these were emitted early,
                # interleaved with the previous layer's hop-3 (see below)
                if g == 0:
                    for qq in range(NQ):
                        for b in range(qq * 25, min(qq * 25 + 25, 98)):
                            s = b // 4
                            if s not in lin0_done:
                                lin0_done.add(s)
                                emit_lin0_slab(s)
                            emit_z0_block(b, par)
                        emit_ag(qq, par)
                    for s in range(NBLK // 4):
                        if s not in lin0_done:
                            lin0_done.add(s)
                            emit_lin0_slab(s)

                # out^T init: W[g,0]^T hT
                for bb in range(0, NBLK, 4):
                    po = pden.tile([128, 512], DT.float32, tag="ph")
                    nc.tensor.matmul(po[:], wtag_sb[g * nm1][:],
                                     hT[:, 128 * bb:128 * bb + 512])
                    nc.vector.tensor_copy(oT[:, 128 * bb:128 * bb + 512], po[:])

                for k in range(1, k_hops + 1):
                    nxt = par ^ 1
                    ag_fired = set()
                    z0_fired = set()
                    def emit_calls(csub, acc, blocks):
                        for (gi_, qq, c_off, L, segs) in csub:
                            msg = mpool.tile([128, MAXL // 128, 128], BF16, tag="msg")
                            nc.gpsimd.dma_gather(
                                out_ap=msg[:, :L // 128, :],
                                in_ap=ztab[par][qq][:, :],
                                idxs_ap=idx_sb[:, c_off // 16:(c_off + L) // 16],
                                num_idxs=L, num_idxs_reg=L, elem_size=128,
                                queue_num=qrr[0] % NQUEUES)
                            qrr[0] += 1
                            cblocks = [b for (b, nch) in segs for _ in range(nch)]
                            nch = L // 128
                            c0 = c_off // 128
                            for q0 in range(0, nch, 4):
                                w = min(4, nch - q0)
                                oh = opool.tile([128, 4, 128], BF16, tag="oh")
                                in0 = iota_h[:].unsqueeze(1).broadcast_to([128, w, 128])
                                in1 = colrel_sb[:, c0 + q0:c0 + q0 + w].unsqueeze(2) \
                                    .broadcast_to([128, w, 128])
                                nc.vector.tensor_tensor(oh[:, :w, :], in0, in1,
                                                        op=mybir.AluOpType.is_equal)
                                for j in range(w):
                                    cj = q0 + j
                                    b = cblocks[cj]
                                    nc.tensor.matmul(
                                        acc[:, b - blocks[0], :], oh[:, j, :],
                                        msg[:, cj, :], start=False,
                                        stop=(c0 + cj == last_chunk[b]))

                    def emit_finalize(blocks, acc):
                        for b in blocks:
                            if b not in chunks_of_block:
                                continue
                            jj = b - blocks[0]
                            if k < k_hops:
                                zr = wpool.tile([128, 128], BF16, tag="zr")
                                nc.scalar.activation(zr[:], acc[:, jj, :], cpy,
                                                     scale=dis2_sb[:, b:b + 1])
                                qq2 = quarter_of_block(b)
                                lb = b - qq2 * 25
                                nc.sync.dma_start(
                                    zin[nxt][qq2][128 * lb:128 * (lb + 1), :], zr[:])
                            xk = wpool.tile([128, 128], DT.float32, tag="xk")
                            nc.vector.tensor_scalar(xk[:], acc[:, jj, :],
                                                    dis_sb[:, b:b + 1], None,
                                                    op0=mybir.AluOpType.mult)
                            aux = paux.tile([128, 2, 128], DT.float32,
                                            name=f"aux_{g}_{k}_{b}", tag="aux")
                            nc.tensor.transpose(aux[:, 0, :], xk[:], ident[:])
                            xkT = wpool.tile([128, 128], BF16, tag="xkT")
                            nc.scalar.activation(xkT[:], aux[:, 0, :], cpy)
                            nc.tensor.matmul(aux[:, 1, :], wtagh_sb[g * nm1 + k][:],
                                             xkT[:])
                            nc.vector.tensor_add(oT[:, 128 * b:128 * (b + 1)],
                                                 oT[:, 128 * b:128 * (b + 1)],
                                                 aux[:, 1, :])
                        if k < k_hops:
                            for qq2 in range(NQ):
                                if qq2 not in ag_fired and blocks[-1] >= qq2 * 25 + 24:
                                    ag_fired.add(qq2)
                                    emit_ag(qq2, nxt)
                        else:
                            bb = blocks[0]
                            nc.scalar.activation(hT[:, 128 * bb:128 * bb + 512],
                                                 oT[:, 128 * bb:128 * bb + 512],
                                                 rel, bias=btag_sb[:, g:g + 1])
                            if g + 1 < n_g:
                                for b in blocks:
                                    if b < 98:
                                        emit_z0_block(b, par ^ 1)
                                for qq2 in range(NQ):
                                    if qq2 not in z0_fired and \
                                       blocks[-1] >= qq2 * 25 + 24:
                                        z0_fired.add(qq2)
                                        emit_ag(qq2, par ^ 1)

                    # group 0 defers its quarter-3 scatter until after group 1,
                    # hiding the previous round's last AllGather tail
                    deferred = None
                    for gi, blocks in enumerate(groups):
                        gc012 = [c for qq in range(NQ - 1)
                                 for c in calls_of_group.get((gi, qq), [])]
                        gc3 = calls_of_group.get((gi, NQ - 1), [])
                        if not (gc012 or gc3):
                            continue
                        acc = pacc.tile([128, GBLK, 128], DT.float32,
                                        name=f"acc_{g}_{k}_{gi}", tag="acc")
                        nc.scalar.memzero(acc[:])
                        if gi == 0:
                            emit_calls(gc012, acc, blocks)
                            deferred = (blocks, acc, gc3)
                            continue
                        emit_calls(gc012 + gc3, acc, blocks)
                        emit_finalize(blocks, acc)
                        if deferred is not None:
                            db, dacc, dq3 = deferred
                            deferred = None
                            emit_calls(dq3, dacc, db)
                            emit_finalize(db, dacc)
                    if k < k_hops:
                        assert ag_fired == set(range(NQ))
                        par = nxt
                if g + 1 < n_g:
                    par ^= 1



            # ---- MLP ----
            for m in range(n_m):
                src_t, dst_t = (hT, oT) if m % 2 == 0 else (oT, hT)
                for bb in range(0, NBLK, 4):
                    po = pden.tile([128, 512], DT.float32, tag="ph")
                    nc.tensor.matmul(po[:], wmlp_sb[m][:],
                                     src_t[:, 128 * bb:128 * bb + 512])
                    nc.scalar.activation(dst_t[:, 128 * bb:128 * bb + 512], po[:],
                                         rel, bias=bmlp_sb[:, m:m + 1])
            hT = oT if n_m % 2 == 1 else hT

            # ---- head ----
            ysb = big.tile([1, NB], DT.float32)
            for bb in range(0, NBLK, 4):
                py = pden.tile([1, 512], DT.float32, tag="ph")
                nc.tensor.matmul(py[:], w1_sb[:], hT[:, 128 * bb:128 * bb + 512])
                nc.scalar.activation(ysb[:, 128 * bb:128 * bb + 512], py[:],
                                     rel, bias=b1_sb[:])
                nc.sync.dma_start(y_d[:, 128 * bb:128 * bb + 512],
                                  ysb[:, 128 * bb:128 * bb + 512])

    nc.compile()
    return nc


def _setup(x, edge_index, W0, b0, W_tag, b_tag, W_mlp, b_mlp, W1, b1):
    x = np.asarray(x, np.float32)
    edge_index = np.asarray(edge_index)
    n_real = x.shape[0]
    n_g, nm1 = W_tag.shape[0], W_tag.shape[1]
    n_m = W_mlp.shape[0]

    ck = (n_real, edge_index.shape[1], int(edge_index[0, ::997].astype(np.int64).sum()),
          int(edge_index[1, ::997].astype(np.int64).sum()))
    if ck not in _cache:
        prep = _host_prep(edge_index, n_real)
        nc = _build(prep, n_g, nm1 - 1, n_m)
        _cache[ck] = (prep, nc)
    prep, nc = _cache[ck]

    npc = prep["npc"]
    xT = np.zeros((P, 8, NB), np.float32)
    xs = x.reshape(P, npc, -1)
    for c in range(P):
        xT[c, :xs.shape[2], :npc] = xs[c].T

    wtag = np.ascontiguousarray(W_tag.reshape(n_g * nm1, 128, 128), dtype=np.float32)
    in_maps = []
    for c in range(P):
        in_maps.append({
            "xT": xT[c], "idx": prep["idx16"][c], "colrel": prep["colrel"][c],
            "dis": prep["dis"][c], "dis2": prep["dis2"][c],
            "w0": np.vstack([np.asarray(W0, np.float32),
                             np.zeros((8 - W0.shape[0], 128), np.float32)]),
            "b0": np.asarray(b0, np.float32).reshape(128, 1),
            "wtag": wtag,
            "wtagh": wtag.astype(ml_dtypes.bfloat16),
            "btag": np.ascontiguousarray(np.asarray(b_tag, np.float32).T),
            "wmlp": np.asarray(W_mlp, np.float32),
            "bmlp": np.ascontiguousarray(np.asarray(b_mlp, np.float32).T),
            "w1": np.asarray(W1, np.float32),
            "b1": np.asarray(b1, np.float32).reshape(1, 1),
        })
    return nc, in_maps, npc, n_real


def kernel(**inputs):
    nc, in_maps, npc, n_real = _setup(**inputs)
    res = run_bass_kernel_spmd(nc, in_maps, list(range(P)))
    out = np.concatenate([res.results[c]["y"][0, :npc] for c in range(P)])
    return out.reshape(n_real, 1).astype(np.float32)


def run_traced(inputs):
    nc, in_maps, npc, n_real = _setup(**inputs)
    return run_bass_kernel_spmd(nc, in_maps, list(range(P)), trace=True)
